# revision 31
# baseline (speedup 1.0000x reference)
"""Trainium2 Bass kernel for nn_DecoderLayer (GNN message passing decoder layer).

Math (per reference):
  seq_j = seq_emb[edge_idx] * ar_mask[..., None]
  x = concat([h_i, h_j, edge_h, seq_j], -1)            # [res,k,4h]
  msg = gelu(x @ mW1 + mb1); msg = gelu(msg @ mW2 + mb2); msg = msg @ mW3 + mb3
  agg = msg.sum(1)
  h = LN(node_h + agg) * g1 + b1
  ff = gelu(h @ fW1 + fb1) @ fW2 + fb2
  h = LN(h + ff) * g2 + b2

Strategy (8-way data parallel over the residue dim, no collectives):
  - mm1 decomposed: x@mW1 = h_i@Wa + h_j@Wb + edge_h@Wc + seq_j@Wd.
    h_j@Wb and seq_emb@Wd are precomputed per global node into a fused FP8
    gather table [8192, 256] in DRAM; per-edge rows fetched with dma_gather
    (256B packets - half the DMA-engine packet cost of bf16).
  - edge_h streamed in fp8 as well (mixed fp8xbf16 matmuls keep the weights
    in bf16, so only per-edge data pays fp8 quantization; predicted rel err
    ~5e-3 vs the 2e-2 gate).
  - dma_gather descriptor generation runs on the GpSimd Q7 core-pair selected
    by queue_num; queues 1-3 run concurrently on three disjoint pairs.
  - Edges are k-major within 3072-edge chunks (64 nodes x 48 k): col=k*64+n.
    Aligns 512-col matmul slices with the per-node h_i@Wa broadcast AP and
    makes the k-reduction a log-tree of dense bf16 adds on DVE.
  - ar_mask folded into the PE transpose of the gathered seq half via
    rhs = diag(mask); the diag tiles are built ON DEVICE per chunk with one
    stride-0 int16 DVE multiply (select) from a 96KB packed mask input --
    replaces the 12.6MB host-built diag tensor of the earlier version.
  - k-reduction before mm3 (linearity): 48x less mm3 work.
  - mm2 (+gelu2+reduce) of chunk N is issued interleaved into chunk N+1's
    mm1 stream so ACT always has ready work and PE never waits on gelu1.
"""

import os
import sys

sys.path.insert(0, "/opt/trn_rl_repo")

import numpy as np
import ml_dtypes

import concourse.bacc as bacc
import concourse.bass as bass
import concourse.mybir as mybir
import concourse.tile as tile
from concourse import bass_utils

BF16 = ml_dtypes.bfloat16
FP8 = ml_dtypes.float8_e4m3
F32 = mybir.dt.float32
BF = mybir.dt.bfloat16
I16 = mybir.dt.int16
F8 = mybir.dt.float8e4

RES, KK, H = 8192, 48, 128
N_CORES = 8
CH_NODES = 64                 # nodes per chunk
CH_E = CH_NODES * KK          # 3072 edges per chunk
HC_E = CH_E // 2              # 1536 edges per half-chunk
N_SUB = CH_E // 128           # 24 subtiles of 128 edges per chunk


def build_nc(n_glob, n_loc, num_devices):
    E = n_loc * KK
    n_ch = E // CH_E           # 16 chunks
    nblk = n_loc // 128        # 8 local node blocks
    gblk = n_glob // 128       # 64 global node blocks

    nc = bacc.Bacc("TRN2", target_bir_lowering=False, debug=False,
                   num_devices=num_devices, num_swdge_queues=4)

    def din(name, shape, dt):
        return nc.dram_tensor(name, shape, dt, kind="ExternalInput")

    edge_hT = din("edge_hT", [H, E], F8)            # k-major per chunk, fp8
    idx16 = din("idx16", [128, E // 16], I16)       # k-major per chunk
    node_hT = din("node_hT", [H, n_glob], BF)       # rotated: local first
    seqT = din("seqT", [H, n_glob], BF)
    blob_bf = din("blob_bf", [128, 2176], BF)       # packed bf16 constants
    blob_f32 = din("blob_f32", [128, 1288], F32)    # packed f32 constants
    id01 = din("id01", [128, 128], I16)             # identity as int 0/1
    # vpk[p, s] = (fp8bits(mask) << 8) | fp8bits(1.0): one u16 per edge whose
    # int16-select against id01 yields the interleaved (1.0, m) fp8 pair diag
    # consumed as the DoubleRow rhs.
    vpk = din("vpk", [128, E // 128], I16)
    out = nc.dram_tensor("out", [n_loc, H], F32, kind="ExternalOutput")

    GELU = mybir.ActivationFunctionType.Gelu
    IDENT = mybir.ActivationFunctionType.Identity
    COPY = mybir.ActivationFunctionType.Copy
    SQRT = mybir.ActivationFunctionType.Sqrt
    SUB = mybir.AluOpType.subtract
    MUL = mybir.AluOpType.mult

    with tile.TileContext(nc) as tc:
        with tc.tile_pool(name="singles", bufs=1) as sg, \
             tc.tile_pool(name="dram", bufs=1, space="DRAM") as dp:
            # ---- resident constants: two packed blobs + idx + mask ----
            s_bb = sg.tile([128, 2176], BF)
            nc.sync.dma_start(out=s_bb[:, 128:384],
                              in_=blob_bf.ap()[:, 128:384])
            nc.sync.dma_start(out=s_bb[:, 0:128], in_=blob_bf.ap()[:, 0:128])
            nc.sync.dma_start(out=s_bb[:, 384:2176],
                              in_=blob_bf.ap()[:, 384:2176])
            s_bf = sg.tile([128, 1288], F32)
            nc.scalar.dma_start(out=s_bf[:], in_=blob_f32.ap())
            s_idx = sg.tile([128, E // 16], I16)
            nc.sync.dma_start(out=s_idx[:], in_=idx16.ap())
            s_id01 = sg.tile([128, 128], I16)
            nc.scalar.dma_start(out=s_id01[:], in_=id01.ap())
            s_vpk = sg.tile([128, E // 128], I16)
            nc.scalar.dma_start(out=s_vpk[:], in_=vpk.ap())
            s_wa = s_bb[:, 0:128]
            s_wb = s_bb[:, 128:256]
            s_wd = s_bb[:, 256:384]
            s_wc = s_bb[:, 384:512]
            s_w2 = s_bb[:, 512:640]
            s_w3 = s_bb[:, 640:768]
            s_fw1 = s_bb[:, 768:1280]
            s_fw2 = s_bb[:, 1280:1792].rearrange("p (a b) -> p a b", a=4)
            s_id = s_bb[:, 1792:1920]
            s_g1bc = s_bb[:, 1920:2048]
            s_b1bc = s_bb[:, 2048:2176]
            s_mb1c = s_bf[:, 0:1]
            s_mb2c = s_bf[:, 1:2]
            s_mb3x48 = s_bf[:, 2:3]
            s_fb1c = s_bf[:, 3:7]
            s_fb2c = s_bf[:, 7:8]
            s_g2bc = s_bf[:, 8:136]
            s_b2bc = s_bf[:, 136:264]
            s_nhl = s_bf[:, 264:1288].rearrange("p (a b) -> p a b", a=nblk)
            s_eps = sg.tile([128, 1], F32)
            nc.vector.memset(s_eps[:], 1e-5)

            s_aT = sg.tile([128, n_loc], BF)        # (Wa^T h_i) per local node
            s_aggTb = sg.tile([128, n_loc], BF)     # k-sum of msg2, fm bf16
            s_a2Tb = sg.tile([128, n_loc], BF)
            s_h1T = sg.tile([128, n_loc], BF)
            s_h1rm = sg.tile([128, nblk, H], BF)

            table = dp.tile([n_glob, 256], F8)

            # ---- phase 1: gather table (fp8) + Wa precompute ----
            # (phase-2 SBUF pools open first so their zone sits below the
            # phase-1 pools on the allocator stack: phase-2 DMAs can then
            # start during phase 1 instead of waiting for its release)
            ctx2 = [tc.tile_pool(name="p2g", bufs=8),
                    tc.tile_pool(name="p2e", bufs=3),
                    tc.tile_pool(name="p2d", bufs=3),
                    tc.tile_pool(name="p2t2", bufs=4),
                    tc.tile_pool(name="p2r", bufs=2),
                    tc.tile_pool(name="p2t4", bufs=3),
                    tc.tile_pool(name="p2x", bufs=1)]
            p2g, p2e, p2d, p2t2, p2r, p2t4, p2x = [c.__enter__() for c in ctx2]
            with tc.tile_pool(name="p1s", bufs=1) as p1s, \
                 tc.tile_pool(name="p1p", bufs=2, space="PSUM") as p1p, \
                 tc.tile_pool(name="p1p2", bufs=3, space="PSUM") as p1p2:
                nhT_bf = p1s.tile([128, n_glob], BF, tag="big1")
                seT_bf = p1s.tile([128, n_glob], BF, tag="big2")
                qn = n_glob // 4
                for qq in range(4):
                    nc.sync.dma_start(out=nhT_bf[:, qn * qq:qn * (qq + 1)],
                                      in_=node_hT.ap()[:, qn * qq:qn * (qq + 1)])
                    nc.scalar.dma_start(out=seT_bf[:, qn * qq:qn * (qq + 1)],
                                        in_=seqT.ap()[:, qn * qq:qn * (qq + 1)])
                # table rows: node-major fp8, built 4 blocks (512 nodes) per
                # PSUM tile so the copy/write pipeline amortizes hop latency.
                tstage = p1s.tile([128, gblk, 256], F8, tag="tstage")
                for gq in range(gblk // 4):
                    ps4 = p1p2.tile([128, 4, 256], F32, tag="tps4")
                    for j in range(4):
                        b = 4 * gq + j
                        nc.tensor.matmul(out=ps4[:, j, 0:128],
                                         lhsT=nhT_bf[:, 128 * b:128 * (b + 1)],
                                         rhs=s_wb[:], start=True, stop=True)
                        nc.tensor.matmul(out=ps4[:, j, 128:256],
                                         lhsT=seT_bf[:, 128 * b:128 * (b + 1)],
                                         rhs=s_wd[:], start=True, stop=True)
                    if gq % 2 == 0:
                        nc.scalar.activation(out=tstage[:, 4 * gq:4 * gq + 4, :],
                                             in_=ps4[:], func=COPY)
                    else:
                        nc.vector.tensor_copy(out=tstage[:, 4 * gq:4 * gq + 4, :],
                                              in_=ps4[:])
                    g0 = 4 * gq
                    tslice = table[128 * g0:128 * (g0 + 4), :]
                    tslice = tslice.rearrange("(b p) f -> p b f", p=128)
                    nc.sync.dma_start(out=tslice,
                                      in_=tstage[:, g0:g0 + 4, :])
                # aT = Wa^T h for local nodes (after the table: the table
                # gates the gathers, aT only gates chunk-0 mm1)
                for hh in range(n_loc // 512):
                    psa = p1p.tile([128, 512], F32, tag="psa")
                    nc.tensor.matmul(out=psa[:], lhsT=s_wa[:],
                                     rhs=nhT_bf[:, 512 * hh:512 * (hh + 1)],
                                     start=True, stop=True)
                    nc.scalar.activation(out=s_aT[:, 512 * hh:512 * (hh + 1)],
                                         in_=psa[:], func=COPY)

            # ---- phase 2: main edge loop, k-major chunks ----
            with tc.tile_pool(name="pp1", bufs=2, space="PSUM") as pp1, \
                 tc.tile_pool(name="ppw", bufs=2, space="PSUM") as ppw:
                def w2_stage(pend, hc):
                    t2s, t4p = pend
                    e0 = HC_E * hc
                    for b in range(3):
                        psw = ppw.tile([128, 512], F32, tag="psw",
                                       name=f"psw{hc}{b}")
                        nc.tensor.matmul(out=psw[:], lhsT=s_w2[:],
                                         rhs=t2s[hc][:, 512 * b:512 * (b + 1)],
                                         start=True, stop=True)
                        nc.scalar.activation(
                            out=t4p[:, e0 + 512 * b:e0 + 512 * (b + 1)],
                            in_=psw[:], func=GELU, bias=s_mb2c[:])

                def reduce_stage(pendx, _unused, chp):
                    # dense log-tree over the k-major layout (DVE; GpSimd's
                    # strict FIFO is owned by the gather instructions)
                    _, t4p = pendx[0]
                    r1 = p2r.tile([128, HC_E], BF, tag="r1")
                    nc.vector.tensor_add(out=r1[:], in0=t4p[:, 0:HC_E],
                                         in1=t4p[:, HC_E:CH_E])
                    nc.vector.tensor_add(out=r1[:, 0:768], in0=r1[:, 0:768],
                                         in1=r1[:, 768:1536])
                    nc.vector.tensor_add(out=r1[:, 0:384], in0=r1[:, 0:384],
                                         in1=r1[:, 384:768])
                    nc.vector.tensor_add(out=r1[:, 0:192], in0=r1[:, 0:192],
                                         in1=r1[:, 192:384])
                    nc.vector.tensor_add(out=r1[:, 0:64], in0=r1[:, 0:64],
                                         in1=r1[:, 64:128])
                    nc.vector.tensor_add(out=s_aggTb[:, CH_NODES * chp:
                                                     CH_NODES * (chp + 1)],
                                         in0=r1[:, 0:64], in1=r1[:, 128:192])

                pend = None
                for ch in range(n_ch):
                    # each chunk's gather split across swdge queues so
                    # descriptor generation runs in parallel; the first two
                    # chunks split 4 ways to cut the pipeline-fill latency
                    g = p2g.tile([128, N_SUB, 256], F8, tag="g")
                    nsplit = 4 if ch < 2 else 2
                    sub_s = N_SUB // nsplit
                    idx_s = (CH_E // 16) // nsplit
                    for hg in range(nsplit):
                        nc.gpsimd.dma_gather(
                            out_ap=g[:, sub_s * hg:sub_s * (hg + 1), :],
                            in_ap=table[:],
                            idxs_ap=s_idx[:, (CH_E // 16) * ch + idx_s * hg:
                                          (CH_E // 16) * ch + idx_s * (hg + 1)],
                            num_idxs=CH_E // nsplit,
                            num_idxs_reg=CH_E // nsplit,
                            elem_size=256,
                            single_packet=False,
                            queue_num=(2 * ch + hg) % 4,
                        )
                    e = p2e.tile([128, CH_E], F8, tag="e")
                    nc.sync.dma_start(out=e[:],
                                      in_=edge_hT.ap()[:, CH_E * ch:
                                                       CH_E * (ch + 1)])
                    # diag for this chunk (int16 select on DVE; GpSimd/Pool
                    # rejects int16 mult):
                    # dia_u16[p, s, n] = id01[p, n] * vpk[p, 24*ch + s];
                    # bitcast as fp8 pairs it is diag((1.0, m)) per subtile.
                    dia = p2d.tile([128, N_SUB, 128], I16, tag="dia")
                    idb = bass.AP(tensor=s_id01.tensor, offset=s_id01.offset,
                                  ap=[s_id01.ap[0], [0, N_SUB], s_id01.ap[1]])
                    vsl = s_vpk[:, N_SUB * ch:N_SUB * (ch + 1)]
                    vb = bass.AP(tensor=vsl.tensor, offset=vsl.offset,
                                 ap=[vsl.ap[0], vsl.ap[1], [0, 128]])
                    nc.vector.tensor_mul(out=dia[:], in0=idb, in1=vb)
                    dg8 = dia[:].bitcast(F8)

                    t4 = p2t4.tile([128, CH_E], BF, tag="t4")
                    na = s_aT[:, CH_NODES * ch:CH_NODES * (ch + 1)]
                    rep = bass.AP(tensor=na.tensor, offset=na.offset,
                                  ap=[na.ap[0], [0, 8], na.ap[1]])
                    t2s = []
                    for hc in range(2):
                        ps1 = pp1.tile([128, 3, 512], F32, tag="ps1")
                        e0 = HC_E * hc  # edge col offset within chunk
                        for b in range(3):
                            nc.tensor.matmul(
                                out=ps1[:, b, :], lhsT=s_wc[:],
                                rhs=e[:, e0 + 512 * b:e0 + 512 * (b + 1)],
                                start=True, stop=False)
                        for b in range(3):
                            nc.tensor.matmul(out=ps1[:, b, :], lhsT=s_id[:],
                                             rhs=rep, start=False, stop=False)
                        for sub in range(12):
                            gsub = 12 * hc + sub
                            bank = sub // 4
                            col = 128 * (sub % 4)
                            # fused DoubleRow: psum += h_j + m * seq_j in one
                            # matmul (planar lhsT pairs, interleaved rhs pairs)
                            gs = g[:, gsub, :]
                            lhsT = bass.AP(tensor=gs.tensor, offset=gs.offset,
                                           ap=[gs.ap[0], [128, 2], [1, 128]])
                            ds = dg8[:, gsub, :]
                            rhsd = bass.AP(tensor=ds.tensor, offset=ds.offset,
                                           ap=[ds.ap[0], [1, 2], [2, 128]])
                            nc.tensor.matmul(
                                out=ps1[:, bank, col:col + 128],
                                lhsT=lhsT, rhs=rhsd,
                                start=False, stop=True,
                                perf_mode=mybir.MatmulPerfMode.DoubleRow)
                        # previous chunk's w2 stage first: its gelu2 inputs
                        # are ready, so ACT drains them while PE works here.
                        if pend is not None:
                            w2_stage(pend[0], hc)
                            if hc == 1:
                                reduce_stage((pend[0],), None, pend[1])
                        t2 = p2t2.tile([128, HC_E], BF, tag="t2")
                        nc.scalar.activation(out=t2[:], in_=ps1[:], func=GELU,
                                             bias=s_mb1c[:])
                        t2s.append(t2)
                    pend = ((t2s, t4), ch)
                    # phase-3 head start for the first 512 nodes (chunks 0-7
                    # fully reduced by iteration 9): mm3, LN1, and the
                    # transpose back ride spare ppw psum slots mid-loop.
                    if ch == 10:
                        psm0 = ppw.tile([128, 512], F32, tag="psw",
                                        name="mm3h0")
                        nc.tensor.matmul(out=psm0[:], lhsT=s_w3[:],
                                         rhs=s_aggTb[:, 0:512],
                                         start=True, stop=True)
                        nc.scalar.activation(out=s_a2Tb[:, 0:512],
                                             in_=psm0[:], func=IDENT,
                                             bias=s_mb3x48[:])
                    if ch == 11:
                        psT0 = ppw.tile([128, 512], F32, tag="psw",
                                        name="lnT0")
                        v0 = psT0[:].rearrange("p (a b) -> p a b", a=4)
                        for b in range(4):
                            nc.tensor.matmul(
                                out=v0[:, b, :],
                                lhsT=s_a2Tb[:, 128 * b:128 * (b + 1)],
                                rhs=s_id[:], start=True, stop=True)
                        x1h = p2x.tile([128, 4, 128], F32, tag="x1h")
                        nc.vector.tensor_add(out=x1h[:], in0=v0,
                                             in1=s_nhl[:, 0:4, :])
                        sth = p2x.tile([128, 4, 6], F32, tag="sth")
                        mvh = p2x.tile([128, 4, 2], F32, tag="mvh")
                        for b in range(4):
                            nc.vector.bn_stats(out=sth[:, b, :],
                                               in_=x1h[:, b, :])
                            nc.vector.bn_aggr(out=mvh[:, b, :],
                                              in_=sth[:, b, :])
                        sdh = p2x.tile([128, 4], F32, tag="sdh")
                        nc.scalar.activation(out=sdh[:], in_=mvh[:, :, 1:2],
                                             func=SQRT, bias=s_eps[:])
                        rsh = p2x.tile([128, 4], F32, tag="rsh")
                        nc.vector.reciprocal(out=rsh[:], in_=sdh[:])
                        xnh = p2x.tile([128, 4, 128], BF, tag="xnh")
                        for b in range(4):
                            nc.vector.tensor_scalar(out=xnh[:, b, :],
                                                    in0=x1h[:, b, :],
                                                    scalar1=mvh[:, b, 0:1],
                                                    scalar2=rsh[:, b:b + 1],
                                                    op0=SUB, op1=MUL)
                        g1r4 = bass.AP(tensor=s_g1bc.tensor,
                                       offset=s_g1bc.offset,
                                       ap=[s_g1bc.ap[0], [0, 4], s_g1bc.ap[1]])
                        b1r4 = bass.AP(tensor=s_b1bc.tensor,
                                       offset=s_b1bc.offset,
                                       ap=[s_b1bc.ap[0], [0, 4], s_b1bc.ap[1]])
                        tbh = p2x.tile([128, 4, 128], BF, tag="tbh")
                        nc.vector.tensor_mul(out=tbh[:], in0=xnh[:], in1=g1r4)
                        nc.vector.tensor_add(out=s_h1rm[:, 0:4, :],
                                             in0=tbh[:], in1=b1r4)
                    if ch == 12:
                        psT1 = ppw.tile([128, 512], F32, tag="psw",
                                        name="h1T0")
                        v1 = psT1[:].rearrange("p (a b) -> p a b", a=4)
                        for b in range(4):
                            nc.tensor.matmul(out=v1[:, b, :],
                                             lhsT=s_h1rm[:, b, :],
                                             rhs=s_id[:], start=True,
                                             stop=True)
                        nc.scalar.activation(out=s_h1T[:, 0:512],
                                             in_=psT1[:], func=COPY)
                w2_stage(pend[0], 0)
                w2_stage(pend[0], 1)
                reduce_stage((pend[0],), None, pend[1])

            for c in reversed(ctx2):
                c.__exit__(None, None, None)

            # ---- phase 3: mm3, LN1, FF, LN2, output (stage-batched) ----
            with tc.tile_pool(name="p3s", bufs=8) as p3s, \
                 tc.tile_pool(name="p3u", bufs=4) as p3u, \
                 tc.tile_pool(name="p3o", bufs=2) as p3o, \
                 tc.tile_pool(name="pp3", bufs=1, space="PSUM") as pp3, \
                 tc.tile_pool(name="ppm", bufs=1, space="PSUM") as ppm, \
                 tc.tile_pool(name="ppf", bufs=2, space="PSUM") as ppf, \
                 tc.tile_pool(name="ppf2", bufs=2, space="PSUM") as ppf2:
                nh_half = n_loc // 512

                def ff_half(hh):
                    us = []
                    for fc in range(4):
                        psf = ppf.tile([128, 512], F32, tag="psf")
                        nc.tensor.matmul(out=psf[:],
                                         lhsT=s_fw1[:, 128 * fc:128 * (fc + 1)],
                                         rhs=s_h1T[:, 512 * hh:512 * (hh + 1)],
                                         start=True, stop=True)
                        u = p3u.tile([128, 512], BF, tag=f"u{fc}")
                        nc.scalar.activation(out=u[:], in_=psf[:], func=GELU,
                                             bias=s_fb1c[:, fc:fc + 1])
                        us.append(u)
                    psf2 = ppf2.tile([128, 512], F32, tag="psf2")
                    for fc in range(4):
                        nc.tensor.matmul(out=psf2[:], lhsT=s_fw2[:, fc, :],
                                         rhs=us[fc][:], start=(fc == 0),
                                         stop=(fc == 3))
                    ffT4 = p3s.tile([128, 4, 128], BF, tag="ffT4")
                    nc.scalar.activation(out=ffT4[:], in_=psf2[:],
                                         func=IDENT, bias=s_fb2c[:])
                    psr2T = pp3.tile([128, 4, 128], F32, tag="psr2T")
                    for j in range(4):
                        nc.tensor.matmul(out=psr2T[:, j, :],
                                         lhsT=ffT4[:, j, :],
                                         rhs=s_id[:], start=True, stop=True)
                    x2a = p3s.tile([128, 4, 128], F32, tag="x2a")
                    nc.vector.tensor_add(out=x2a[:], in0=psr2T[:],
                                         in1=s_h1rm[:, 4 * hh:4 * hh + 4, :])
                    st2a = p3s.tile([128, 4, 6], F32, tag="st2a")
                    mv2a = p3s.tile([128, 4, 2], F32, tag="mv2a")
                    for j in range(4):
                        nc.vector.bn_stats(out=st2a[:, j, :], in_=x2a[:, j, :])
                        nc.vector.bn_aggr(out=mv2a[:, j, :], in_=st2a[:, j, :])
                    sd2a = p3s.tile([128, 4], F32, tag="sd2a")
                    nc.scalar.activation(out=sd2a[:], in_=mv2a[:, :, 1:2],
                                         func=SQRT, bias=s_eps[:])
                    rstd2a = p3s.tile([128, 4], F32, tag="rstd2a")
                    nc.vector.reciprocal(out=rstd2a[:], in_=sd2a[:])
                    xn2a = p3s.tile([128, 4, 128], F32, tag="xn2a")
                    for j in range(4):
                        nc.vector.tensor_scalar(out=xn2a[:, j, :],
                                                in0=x2a[:, j, :],
                                                scalar1=mv2a[:, j, 0:1],
                                                scalar2=rstd2a[:, j:j + 1],
                                                op0=SUB, op1=MUL)
                    g2rep = bass.AP(tensor=s_g2bc.tensor, offset=s_g2bc.offset,
                                    ap=[s_g2bc.ap[0], [0, 4], s_g2bc.ap[1]])
                    b2rep = bass.AP(tensor=s_b2bc.tensor, offset=s_b2bc.offset,
                                    ap=[s_b2bc.ap[0], [0, 4], s_b2bc.ap[1]])
                    ob = p3o.tile([128, 4, 128], F32, tag="ob")
                    tga = p3s.tile([128, 4, 128], F32, tag="tga")
                    nc.vector.tensor_mul(out=tga[:], in0=xn2a[:], in1=g2rep)
                    nc.vector.tensor_add(out=ob[:], in0=tga[:], in1=b2rep)
                    oslice = out.ap()[512 * hh:512 * (hh + 1), :]
                    oslice = oslice.rearrange("(j p) f -> p j f", p=128)
                    nc.sync.dma_start(out=oslice, in_=ob[:])

                # FF half-0 first: its h1T was produced inside the phase-2
                # loop, so it overlaps the half-1 mm3/LN1 chain below.
                ff_half(0)

                # mm3 + LN1 + transpose-back for the SECOND 512-node half
                psm = ppm.tile([128, 512], F32, tag="psm")
                nc.tensor.matmul(out=psm[:], lhsT=s_w3[:],
                                 rhs=s_aggTb[:, 512:1024],
                                 start=True, stop=True)
                nc.scalar.activation(out=s_a2Tb[:, 512:1024],
                                     in_=psm[:], func=IDENT,
                                     bias=s_mb3x48[:])
                pstT = pp3.tile([128, 4, 128], F32, tag="pstT")
                for b in range(4):
                    nc.tensor.matmul(
                        out=pstT[:, b, :],
                        lhsT=s_a2Tb[:, 512 + 128 * b:512 + 128 * (b + 1)],
                        rhs=s_id[:], start=True, stop=True)
                x1a = p3s.tile([128, 4, 128], F32, tag="x1a")
                nc.vector.tensor_add(out=x1a[:], in0=pstT[:],
                                     in1=s_nhl[:, 4:8, :])
                sta = p3s.tile([128, 4, 6], F32, tag="sta")
                mva = p3s.tile([128, 4, 2], F32, tag="mva")
                for b in range(4):
                    nc.vector.bn_stats(out=sta[:, b, :], in_=x1a[:, b, :])
                    nc.vector.bn_aggr(out=mva[:, b, :], in_=sta[:, b, :])
                sda = p3s.tile([128, 4], F32, tag="sda")
                nc.scalar.activation(out=sda[:], in_=mva[:, :, 1:2],
                                     func=SQRT, bias=s_eps[:])
                rstda = p3s.tile([128, 4], F32, tag="rstda")
                nc.vector.reciprocal(out=rstda[:], in_=sda[:])
                xna = p3s.tile([128, 4, 128], BF, tag="xna")
                for b in range(4):
                    nc.vector.tensor_scalar(out=xna[:, b, :],
                                            in0=x1a[:, b, :],
                                            scalar1=mva[:, b, 0:1],
                                            scalar2=rstda[:, b:b + 1],
                                            op0=SUB, op1=MUL)
                g1rep = bass.AP(tensor=s_g1bc.tensor, offset=s_g1bc.offset,
                                ap=[s_g1bc.ap[0], [0, 4], s_g1bc.ap[1]])
                b1rep = bass.AP(tensor=s_b1bc.tensor, offset=s_b1bc.offset,
                                ap=[s_b1bc.ap[0], [0, 4], s_b1bc.ap[1]])
                tb1a = p3s.tile([128, 4, 128], BF, tag="tb1a")
                nc.vector.tensor_mul(out=tb1a[:], in0=xna[:], in1=g1rep)
                nc.vector.tensor_add(out=s_h1rm[:, 4:8, :], in0=tb1a[:],
                                     in1=b1rep)
                psTT = pp3.tile([128, 4, 128], F32, tag="pstT", name="psTT2")
                for b in range(4):
                    nc.tensor.matmul(out=psTT[:, b, :],
                                     lhsT=s_h1rm[:, 4 + b, :],
                                     rhs=s_id[:], start=True, stop=True)
                nc.scalar.activation(out=s_h1T[:, 512:1024],
                                     in_=psTT[:], func=COPY)
                # FF + LN2 for the second half
                ff_half(1)

    nc.compile()
    return nc


def prep_core_inputs(inputs, n_glob, n_loc, core):
    """Host-side layout prep for one core: slicing, k-major reorder within
    chunks, transposes, dtype casts, tiny constant broadcasts. No kernel math
    (no indexing of data tensors by edge_idx) is done on the host."""
    f32 = np.float32
    n0 = core * n_loc
    E = n_loc * KK
    n_ch = E // CH_E

    def kmajor(x):
        # x: [n_loc, KK, ...] -> per 64-node chunk: [KK, 64, ...] -> flat E
        tail = x.shape[2:]
        x = x.reshape(n_ch, CH_NODES, KK, *tail)
        x = x.transpose(0, 2, 1, *range(3, 3 + len(tail)))
        return np.ascontiguousarray(x.reshape(E, *tail))

    eh = kmajor(inputs["edge_h"][n0:n0 + n_loc])          # [E, H] k-major
    eh = np.ascontiguousarray(eh.T).astype(FP8)           # [H, E] fp8
    j = (inputs["edge_idx"][n0:n0 + n_loc].astype(np.int64) - n0) % n_glob
    j = kmajor(j)                                         # [E] k-major
    idx16 = np.tile(np.ascontiguousarray(j.reshape(E // 16, 16).T), (8, 1)
                    ).astype(np.int16)
    m = kmajor(inputs["ar_mask"][n0:n0 + n_loc])          # [E] k-major
    m8 = np.ascontiguousarray(m.reshape(E // 128, 128).T).astype(FP8)
    one8 = np.float32(1.0).astype(FP8)
    vpk = ((m8.view(np.uint8).astype(np.uint16) << 8)
           | np.uint16(one8.view(np.uint8))).view(np.int16)
    node_hT = np.ascontiguousarray(
        np.roll(inputs["node_h"], -n0, axis=0).T).astype(BF16)
    seqT = np.ascontiguousarray(
        np.roll(inputs["seq_emb"], -n0, axis=0).T).astype(BF16)
    nhl = np.ascontiguousarray(
        inputs["node_h"][n0:n0 + n_loc].reshape(n_loc // 128, 128, H)
        .transpose(1, 0, 2)).astype(f32)
    mW1 = inputs["mW1"]
    bb = np.zeros((128, 2176), dtype=BF16)
    bb[:, 0:128] = mW1[0:128].astype(BF16)
    bb[:, 128:256] = mW1[128:256].astype(BF16)
    bb[:, 256:384] = mW1[384:512].astype(BF16)
    bb[:, 384:512] = mW1[256:384].astype(BF16)
    bb[:, 512:640] = inputs["mW2"].astype(BF16)
    bb[:, 640:768] = inputs["mW3"].astype(BF16)
    bb[:, 768:1280] = inputs["fW1"].astype(BF16)
    bb[:, 1280:1792] = np.ascontiguousarray(
        inputs["fW2"].reshape(4, 128, H).transpose(1, 0, 2)).reshape(
        128, 512).astype(BF16)
    bb[:, 1792:1920] = np.eye(128, dtype=BF16)
    bb[:, 1920:2048] = np.tile(inputs["g1"][None, :], (128, 1)).astype(BF16)
    bb[:, 2048:2176] = np.tile(inputs["b1"][None, :], (128, 1)).astype(BF16)
    bf = np.zeros((128, 1288), dtype=f32)
    bf[:, 0] = inputs["mb1"].astype(f32)
    bf[:, 1] = inputs["mb2"].astype(f32)
    bf[:, 2] = (inputs["mb3"] * KK).astype(f32)
    bf[:, 3:7] = np.ascontiguousarray(inputs["fb1"].reshape(4, 128).T)
    bf[:, 7] = inputs["fb2"].astype(f32)
    bf[:, 8:136] = np.tile(inputs["g2"][None, :], (128, 1))
    bf[:, 136:264] = np.tile(inputs["b2"][None, :], (128, 1))
    bf[:, 264:1288] = nhl.reshape(128, 1024)
    d = {
        "edge_hT": eh, "idx16": idx16,
        "node_hT": node_hT, "seqT": seqT,
        "blob_bf": bb, "blob_f32": bf,
        "id01": np.eye(128, dtype=np.int16), "vpk": vpk,
    }
    return d


_NC_CACHE = {}


def kernel(**inputs):
    inputs = {k: np.asarray(v) for k, v in inputs.items()}
    n_glob = inputs["node_h"].shape[0]
    n_loc = n_glob // N_CORES
    key = (n_glob, n_loc)
    if key not in _NC_CACHE:
        _NC_CACHE[key] = build_nc(n_glob, n_loc, N_CORES)
    nc = _NC_CACHE[key]
    in_maps = [prep_core_inputs(inputs, n_glob, n_loc, c)
               for c in range(N_CORES)]
    res = bass_utils.run_bass_kernel_spmd(nc, in_maps,
                                          core_ids=list(range(N_CORES)))
    return np.concatenate([res.results[c]["out"] for c in range(N_CORES)],
                          axis=0).astype(np.float32)


# revision 33
# speedup vs baseline: 1.1817x; 1.1817x over previous
"""Trainium2 Bass kernel for nn_DecoderLayer (GNN message passing decoder layer).

Math (per reference):
  seq_j = seq_emb[edge_idx] * ar_mask[..., None]
  x = concat([h_i, h_j, edge_h, seq_j], -1)            # [res,k,4h]
  msg = gelu(x @ mW1 + mb1); msg = gelu(msg @ mW2 + mb2); msg = msg @ mW3 + mb3
  agg = msg.sum(1)
  h = LN(node_h + agg) * g1 + b1
  ff = gelu(h @ fW1 + fb1) @ fW2 + fb2
  h = LN(h + ff) * g2 + b2

Strategy (8-way data parallel over the residue dim, no collectives):
  - mm1 decomposed: x@mW1 = h_i@Wa + h_j@Wb + edge_h@Wc + seq_j@Wd.
    h_j@Wb and seq_emb@Wd are precomputed per global node into a fused FP8
    gather table [8192, 256] in DRAM; per-edge rows fetched with dma_gather
    (256B packets - half the DMA-engine packet cost of bf16).
  - edge_h streamed in fp8 as well (mixed fp8xbf16 matmuls keep the weights
    in bf16, so only per-edge data pays fp8 quantization; predicted rel err
    ~5e-3 vs the 2e-2 gate).
  - dma_gather descriptor generation runs on the GpSimd Q7 core-pair selected
    by queue_num; queues 1-3 run concurrently on three disjoint pairs.
  - Edges are k-major within 3072-edge chunks (64 nodes x 48 k): col=k*64+n.
    Aligns 512-col matmul slices with the per-node h_i@Wa broadcast AP and
    makes the k-reduction a log-tree of dense bf16 adds on DVE.
  - ar_mask folded into the PE transpose of the gathered seq half via
    rhs = diag(mask); the diag tiles are built ON DEVICE per chunk with one
    stride-0 int16 DVE multiply (select) from a 96KB packed mask input --
    replaces the 12.6MB host-built diag tensor of the earlier version.
  - k-reduction before mm3 (linearity): 48x less mm3 work.
  - mm2 (+gelu2+reduce) of chunk N is issued interleaved into chunk N+1's
    mm1 stream so ACT always has ready work and PE never waits on gelu1.
"""

import os
import sys

sys.path.insert(0, "/opt/trn_rl_repo")

import numpy as np
import ml_dtypes

import concourse.bacc as bacc
import concourse.bass as bass
import concourse.mybir as mybir
import concourse.tile as tile
from concourse import bass_utils

BF16 = ml_dtypes.bfloat16
FP8 = ml_dtypes.float8_e4m3
F32 = mybir.dt.float32
BF = mybir.dt.bfloat16
I16 = mybir.dt.int16
F8 = mybir.dt.float8e4

RES, KK, H = 8192, 48, 128
N_CORES = 8
CH_NODES = 64                 # nodes per chunk
CH_E = CH_NODES * KK          # 3072 edges per chunk
HC_E = CH_E // 2              # 1536 edges per half-chunk
N_SUB = CH_E // 128           # 24 subtiles of 128 edges per chunk


def build_nc(n_glob, n_loc, num_devices):
    E = n_loc * KK
    n_ch = E // CH_E           # 16 chunks
    nblk = n_loc // 128        # 8 local node blocks
    gblk = n_glob // 128       # 64 global node blocks

    nc = bacc.Bacc("TRN2", target_bir_lowering=False, debug=False,
                   num_devices=num_devices, num_swdge_queues=4)

    def din(name, shape, dt):
        return nc.dram_tensor(name, shape, dt, kind="ExternalInput")

    edge_hT = din("edge_hT", [H, E], F8)            # k-major per chunk, fp8
    idx16 = din("idx16", [128, E // 16], I16)       # k-major per chunk
    node_hT = din("node_hT", [H, n_glob], BF)       # rotated: local first
    seqT = din("seqT", [H, n_glob], BF)
    blob_bf = din("blob_bf", [128, 2176], BF)       # packed bf16 constants
    blob_f32 = din("blob_f32", [128, 1288], F32)    # packed f32 constants
    id01 = din("id01", [128, 128], I16)             # identity as int 0/1
    # vpk[p, s] = (fp8bits(mask) << 8) | fp8bits(1.0): one u16 per edge whose
    # int16-select against id01 yields the interleaved (1.0, m) fp8 pair diag
    # consumed as the DoubleRow rhs.
    vpk = din("vpk", [128, E // 128], I16)
    out = nc.dram_tensor("out", [n_loc, H], F32, kind="ExternalOutput")

    GELU = mybir.ActivationFunctionType.Gelu
    IDENT = mybir.ActivationFunctionType.Identity
    COPY = mybir.ActivationFunctionType.Copy
    SQRT = mybir.ActivationFunctionType.Sqrt
    SUB = mybir.AluOpType.subtract
    MUL = mybir.AluOpType.mult

    with tile.TileContext(nc) as tc:
        with tc.tile_pool(name="singles", bufs=1) as sg, \
             tc.tile_pool(name="dram", bufs=1, space="DRAM") as dp:
            # ---- resident constants: two packed blobs + idx + mask ----
            s_bb = sg.tile([128, 2176], BF)
            nc.sync.dma_start(out=s_bb[:], in_=blob_bf.ap())
            s_bf = sg.tile([128, 1288], F32)
            nc.scalar.dma_start(out=s_bf[:], in_=blob_f32.ap())
            s_idx = sg.tile([128, E // 16], I16)
            nc.sync.dma_start(out=s_idx[:], in_=idx16.ap())
            s_id01 = sg.tile([128, 128], I16)
            nc.scalar.dma_start(out=s_id01[:], in_=id01.ap())
            s_vpk = sg.tile([128, E // 128], I16)
            nc.scalar.dma_start(out=s_vpk[:], in_=vpk.ap())
            s_wa = s_bb[:, 0:128]
            s_wb = s_bb[:, 128:256]
            s_wd = s_bb[:, 256:384]
            s_wc = s_bb[:, 384:512]
            s_w2 = s_bb[:, 512:640]
            s_w3 = s_bb[:, 640:768]
            s_fw1 = s_bb[:, 768:1280]
            s_fw2 = s_bb[:, 1280:1792].rearrange("p (a b) -> p a b", a=4)
            s_id = s_bb[:, 1792:1920]
            s_g1bc = s_bb[:, 1920:2048]
            s_b1bc = s_bb[:, 2048:2176]
            s_mb1c = s_bf[:, 0:1]
            s_mb2c = s_bf[:, 1:2]
            s_mb3x48 = s_bf[:, 2:3]
            s_fb1c = s_bf[:, 3:7]
            s_fb2c = s_bf[:, 7:8]
            s_g2bc = s_bf[:, 8:136]
            s_b2bc = s_bf[:, 136:264]
            s_nhl = s_bf[:, 264:1288].rearrange("p (a b) -> p a b", a=nblk)
            s_eps = sg.tile([128, 1], F32)
            nc.vector.memset(s_eps[:], 1e-5)

            s_aT = sg.tile([128, n_loc], BF)        # (Wa^T h_i) per local node
            s_aggTb = sg.tile([128, n_loc], BF)     # k-sum of msg2, fm bf16
            s_a2Tb = sg.tile([128, n_loc], BF)
            s_h1T = sg.tile([128, n_loc], BF)
            s_h1rm = sg.tile([128, nblk, H], BF)

            table = dp.tile([n_glob, 256], F8)

            # ---- phase 1: gather table (fp8) + Wa precompute ----
            # (phase-2 SBUF pools open first so their zone sits below the
            # phase-1 pools on the allocator stack: phase-2 DMAs can then
            # start during phase 1 instead of waiting for its release)
            ctx2 = [tc.tile_pool(name="p2g", bufs=6),
                    tc.tile_pool(name="p2e", bufs=3),
                    tc.tile_pool(name="p2d", bufs=3),
                    tc.tile_pool(name="p2t2", bufs=4),
                    tc.tile_pool(name="p2r", bufs=2),
                    tc.tile_pool(name="p2t4", bufs=3),
                    tc.tile_pool(name="p2x", bufs=1)]
            p2g, p2e, p2d, p2t2, p2r, p2t4, p2x = [c.__enter__() for c in ctx2]
            with tc.tile_pool(name="p1s", bufs=1) as p1s, \
                 tc.tile_pool(name="p1p", bufs=2, space="PSUM") as p1p, \
                 tc.tile_pool(name="p1p2", bufs=3, space="PSUM") as p1p2:
                nhT_bf = p1s.tile([128, n_glob], BF, tag="big1")
                seT_bf = p1s.tile([128, n_glob], BF, tag="big2")
                qn = n_glob // 4
                for qq in range(4):
                    nc.sync.dma_start(out=nhT_bf[:, qn * qq:qn * (qq + 1)],
                                      in_=node_hT.ap()[:, qn * qq:qn * (qq + 1)])
                    nc.scalar.dma_start(out=seT_bf[:, qn * qq:qn * (qq + 1)],
                                        in_=seqT.ap()[:, qn * qq:qn * (qq + 1)])
                # table rows: node-major fp8, built 4 blocks (512 nodes) per
                # PSUM tile so the copy/write pipeline amortizes hop latency.
                tstage = p1s.tile([128, gblk, 256], F8, tag="tstage")
                for gq in range(gblk // 4):
                    ps4 = p1p2.tile([128, 4, 256], F32, tag="tps4")
                    for j in range(4):
                        b = 4 * gq + j
                        nc.tensor.matmul(out=ps4[:, j, 0:128],
                                         lhsT=nhT_bf[:, 128 * b:128 * (b + 1)],
                                         rhs=s_wb[:], start=True, stop=True)
                        nc.tensor.matmul(out=ps4[:, j, 128:256],
                                         lhsT=seT_bf[:, 128 * b:128 * (b + 1)],
                                         rhs=s_wd[:], start=True, stop=True)
                    if gq % 2 == 0:
                        nc.scalar.activation(out=tstage[:, 4 * gq:4 * gq + 4, :],
                                             in_=ps4[:], func=COPY)
                    else:
                        nc.vector.tensor_copy(out=tstage[:, 4 * gq:4 * gq + 4, :],
                                              in_=ps4[:])
                    if gq % 2 == 1:
                        g0 = 4 * (gq - 1)
                        tslice = table[128 * g0:128 * (g0 + 8), :]
                        tslice = tslice.rearrange("(b p) f -> p b f", p=128)
                        nc.sync.dma_start(out=tslice,
                                          in_=tstage[:, g0:g0 + 8, :])
                # aT = Wa^T h for local nodes (after the table: the table
                # gates the gathers, aT only gates chunk-0 mm1)
                for hh in range(n_loc // 512):
                    psa = p1p.tile([128, 512], F32, tag="psa")
                    nc.tensor.matmul(out=psa[:], lhsT=s_wa[:],
                                     rhs=nhT_bf[:, 512 * hh:512 * (hh + 1)],
                                     start=True, stop=True)
                    nc.scalar.activation(out=s_aT[:, 512 * hh:512 * (hh + 1)],
                                         in_=psa[:], func=COPY)

            # ---- phase 2: main edge loop, k-major chunks ----
            with tc.tile_pool(name="pp1", bufs=2, space="PSUM") as pp1, \
                 tc.tile_pool(name="ppw", bufs=2, space="PSUM") as ppw:
                def w2_stage(pend, hc):
                    t2s, t4p = pend
                    e0 = HC_E * hc
                    for b in range(3):
                        psw = ppw.tile([128, 512], F32, tag="psw",
                                       name=f"psw{hc}{b}")
                        nc.tensor.matmul(out=psw[:], lhsT=s_w2[:],
                                         rhs=t2s[hc][:, 512 * b:512 * (b + 1)],
                                         start=True, stop=True)
                        nc.scalar.activation(
                            out=t4p[:, e0 + 512 * b:e0 + 512 * (b + 1)],
                            in_=psw[:], func=GELU, bias=s_mb2c[:])

                def reduce_stage(pendx, _unused, chp):
                    # dense log-tree over the k-major layout (DVE; GpSimd's
                    # strict FIFO is owned by the gather instructions)
                    _, t4p = pendx[0]
                    r1 = p2r.tile([128, HC_E], BF, tag="r1")
                    nc.vector.tensor_add(out=r1[:], in0=t4p[:, 0:HC_E],
                                         in1=t4p[:, HC_E:CH_E])
                    nc.vector.tensor_add(out=r1[:, 0:768], in0=r1[:, 0:768],
                                         in1=r1[:, 768:1536])
                    nc.vector.tensor_add(out=r1[:, 0:384], in0=r1[:, 0:384],
                                         in1=r1[:, 384:768])
                    nc.vector.tensor_add(out=r1[:, 0:192], in0=r1[:, 0:192],
                                         in1=r1[:, 192:384])
                    nc.vector.tensor_add(out=r1[:, 0:64], in0=r1[:, 0:64],
                                         in1=r1[:, 64:128])
                    nc.vector.tensor_add(out=s_aggTb[:, CH_NODES * chp:
                                                     CH_NODES * (chp + 1)],
                                         in0=r1[:, 0:64], in1=r1[:, 128:192])

                pend = None
                for ch in range(n_ch):
                    # each chunk's gather split across swdge queues so
                    # descriptor generation runs in parallel; the first two
                    # chunks split 4 ways to cut the pipeline-fill latency
                    g = p2g.tile([128, N_SUB, 256], F8, tag="g")
                    nsplit = 4 if ch < 2 else 2
                    sub_s = N_SUB // nsplit
                    idx_s = (CH_E // 16) // nsplit
                    for hg in range(nsplit):
                        nc.gpsimd.dma_gather(
                            out_ap=g[:, sub_s * hg:sub_s * (hg + 1), :],
                            in_ap=table[:],
                            idxs_ap=s_idx[:, (CH_E // 16) * ch + idx_s * hg:
                                          (CH_E // 16) * ch + idx_s * (hg + 1)],
                            num_idxs=CH_E // nsplit,
                            num_idxs_reg=CH_E // nsplit,
                            elem_size=256,
                            single_packet=False,
                            queue_num=(2 * ch + hg) % 4,
                        )
                    e = p2e.tile([128, CH_E], F8, tag="e")
                    nc.sync.dma_start(out=e[:],
                                      in_=edge_hT.ap()[:, CH_E * ch:
                                                       CH_E * (ch + 1)])
                    # diag for this chunk (int16 select on DVE; GpSimd/Pool
                    # rejects int16 mult):
                    # dia_u16[p, s, n] = id01[p, n] * vpk[p, 24*ch + s];
                    # bitcast as fp8 pairs it is diag((1.0, m)) per subtile.
                    dia = p2d.tile([128, N_SUB, 128], I16, tag="dia")
                    idb = bass.AP(tensor=s_id01.tensor, offset=s_id01.offset,
                                  ap=[s_id01.ap[0], [0, N_SUB], s_id01.ap[1]])
                    vsl = s_vpk[:, N_SUB * ch:N_SUB * (ch + 1)]
                    vb = bass.AP(tensor=vsl.tensor, offset=vsl.offset,
                                 ap=[vsl.ap[0], vsl.ap[1], [0, 128]])
                    nc.vector.tensor_mul(out=dia[:], in0=idb, in1=vb)
                    dg8 = dia[:].bitcast(F8)

                    t4 = p2t4.tile([128, CH_E], BF, tag="t4")
                    na = s_aT[:, CH_NODES * ch:CH_NODES * (ch + 1)]
                    rep = bass.AP(tensor=na.tensor, offset=na.offset,
                                  ap=[na.ap[0], [0, 8], na.ap[1]])
                    t2s = []
                    for hc in range(2):
                        ps1 = pp1.tile([128, 3, 512], F32, tag="ps1")
                        e0 = HC_E * hc  # edge col offset within chunk
                        for b in range(3):
                            nc.tensor.matmul(
                                out=ps1[:, b, :], lhsT=s_wc[:],
                                rhs=e[:, e0 + 512 * b:e0 + 512 * (b + 1)],
                                start=True, stop=False)
                        for b in range(3):
                            nc.tensor.matmul(out=ps1[:, b, :], lhsT=s_id[:],
                                             rhs=rep, start=False, stop=False)
                        for sub in range(12):
                            gsub = 12 * hc + sub
                            bank = sub // 4
                            col = 128 * (sub % 4)
                            # fused DoubleRow: psum += h_j + m * seq_j in one
                            # matmul (planar lhsT pairs, interleaved rhs pairs)
                            gs = g[:, gsub, :]
                            lhsT = bass.AP(tensor=gs.tensor, offset=gs.offset,
                                           ap=[gs.ap[0], [128, 2], [1, 128]])
                            ds = dg8[:, gsub, :]
                            rhsd = bass.AP(tensor=ds.tensor, offset=ds.offset,
                                           ap=[ds.ap[0], [1, 2], [2, 128]])
                            nc.tensor.matmul(
                                out=ps1[:, bank, col:col + 128],
                                lhsT=lhsT, rhs=rhsd,
                                start=False, stop=True,
                                perf_mode=mybir.MatmulPerfMode.DoubleRow)
                        # previous chunk's w2 stage first: its gelu2 inputs
                        # are ready, so ACT drains them while PE works here.
                        if pend is not None:
                            w2_stage(pend[0], hc)
                            if hc == 1:
                                reduce_stage((pend[0],), None, pend[1])
                        t2 = p2t2.tile([128, HC_E], BF, tag="t2")
                        nc.scalar.activation(out=t2[:], in_=ps1[:], func=GELU,
                                             bias=s_mb1c[:])
                        t2s.append(t2)
                    pend = ((t2s, t4), ch)
                    # phase-3 head start for the first 512 nodes (chunks 0-7
                    # fully reduced by iteration 9): mm3, LN1, and the
                    # transpose back ride spare ppw psum slots mid-loop.
                    if ch == 10:
                        psm0 = ppw.tile([128, 512], F32, tag="psw",
                                        name="mm3h0")
                        nc.tensor.matmul(out=psm0[:], lhsT=s_w3[:],
                                         rhs=s_aggTb[:, 0:512],
                                         start=True, stop=True)
                        nc.scalar.activation(out=s_a2Tb[:, 0:512],
                                             in_=psm0[:], func=IDENT,
                                             bias=s_mb3x48[:])
                    if ch == 11:
                        psT0 = ppw.tile([128, 512], F32, tag="psw",
                                        name="lnT0")
                        v0 = psT0[:].rearrange("p (a b) -> p a b", a=4)
                        for b in range(4):
                            nc.tensor.matmul(
                                out=v0[:, b, :],
                                lhsT=s_a2Tb[:, 128 * b:128 * (b + 1)],
                                rhs=s_id[:], start=True, stop=True)
                        x1h = p2x.tile([128, 4, 128], F32, tag="x1h")
                        nc.vector.tensor_add(out=x1h[:], in0=v0,
                                             in1=s_nhl[:, 0:4, :])
                        sth = p2x.tile([128, 4, 6], F32, tag="sth")
                        mvh = p2x.tile([128, 4, 2], F32, tag="mvh")
                        for b in range(4):
                            nc.vector.bn_stats(out=sth[:, b, :],
                                               in_=x1h[:, b, :])
                            nc.vector.bn_aggr(out=mvh[:, b, :],
                                              in_=sth[:, b, :])
                        sdh = p2x.tile([128, 4], F32, tag="sdh")
                        nc.scalar.activation(out=sdh[:], in_=mvh[:, :, 1:2],
                                             func=SQRT, bias=s_eps[:])
                        rsh = p2x.tile([128, 4], F32, tag="rsh")
                        nc.vector.reciprocal(out=rsh[:], in_=sdh[:])
                        xnh = p2x.tile([128, 4, 128], BF, tag="xnh")
                        for b in range(4):
                            nc.vector.tensor_scalar(out=xnh[:, b, :],
                                                    in0=x1h[:, b, :],
                                                    scalar1=mvh[:, b, 0:1],
                                                    scalar2=rsh[:, b:b + 1],
                                                    op0=SUB, op1=MUL)
                        g1r4 = bass.AP(tensor=s_g1bc.tensor,
                                       offset=s_g1bc.offset,
                                       ap=[s_g1bc.ap[0], [0, 4], s_g1bc.ap[1]])
                        b1r4 = bass.AP(tensor=s_b1bc.tensor,
                                       offset=s_b1bc.offset,
                                       ap=[s_b1bc.ap[0], [0, 4], s_b1bc.ap[1]])
                        tbh = p2x.tile([128, 4, 128], BF, tag="tbh")
                        nc.vector.tensor_mul(out=tbh[:], in0=xnh[:], in1=g1r4)
                        nc.vector.tensor_add(out=s_h1rm[:, 0:4, :],
                                             in0=tbh[:], in1=b1r4)
                    if ch == 12:
                        psT1 = ppw.tile([128, 512], F32, tag="psw",
                                        name="h1T0")
                        v1 = psT1[:].rearrange("p (a b) -> p a b", a=4)
                        for b in range(4):
                            nc.tensor.matmul(out=v1[:, b, :],
                                             lhsT=s_h1rm[:, b, :],
                                             rhs=s_id[:], start=True,
                                             stop=True)
                        nc.scalar.activation(out=s_h1T[:, 0:512],
                                             in_=psT1[:], func=COPY)
                w2_stage(pend[0], 0)
                w2_stage(pend[0], 1)
                reduce_stage((pend[0],), None, pend[1])

            for c in reversed(ctx2):
                c.__exit__(None, None, None)

            # ---- phase 3: mm3, LN1, FF, LN2, output (stage-batched) ----
            with tc.tile_pool(name="p3s", bufs=8) as p3s, \
                 tc.tile_pool(name="p3u", bufs=4) as p3u, \
                 tc.tile_pool(name="p3o", bufs=2) as p3o, \
                 tc.tile_pool(name="pp3", bufs=1, space="PSUM") as pp3, \
                 tc.tile_pool(name="ppm", bufs=1, space="PSUM") as ppm, \
                 tc.tile_pool(name="ppf", bufs=2, space="PSUM") as ppf, \
                 tc.tile_pool(name="ppf2", bufs=2, space="PSUM") as ppf2:
                nh_half = n_loc // 512

                def ff_half(hh):
                    us = []
                    for fc in range(4):
                        psf = ppf.tile([128, 512], F32, tag="psf")
                        nc.tensor.matmul(out=psf[:],
                                         lhsT=s_fw1[:, 128 * fc:128 * (fc + 1)],
                                         rhs=s_h1T[:, 512 * hh:512 * (hh + 1)],
                                         start=True, stop=True)
                        u = p3u.tile([128, 512], BF, tag=f"u{fc}")
                        nc.scalar.activation(out=u[:], in_=psf[:], func=GELU,
                                             bias=s_fb1c[:, fc:fc + 1])
                        us.append(u)
                    psf2 = ppf2.tile([128, 512], F32, tag="psf2")
                    for fc in range(4):
                        nc.tensor.matmul(out=psf2[:], lhsT=s_fw2[:, fc, :],
                                         rhs=us[fc][:], start=(fc == 0),
                                         stop=(fc == 3))
                    ffT4 = p3s.tile([128, 4, 128], BF, tag="ffT4")
                    nc.scalar.activation(out=ffT4[:], in_=psf2[:],
                                         func=IDENT, bias=s_fb2c[:])
                    psr2T = pp3.tile([128, 4, 128], F32, tag="psr2T")
                    for j in range(4):
                        nc.tensor.matmul(out=psr2T[:, j, :],
                                         lhsT=ffT4[:, j, :],
                                         rhs=s_id[:], start=True, stop=True)
                    x2a = p3s.tile([128, 4, 128], F32, tag="x2a")
                    nc.vector.tensor_add(out=x2a[:], in0=psr2T[:],
                                         in1=s_h1rm[:, 4 * hh:4 * hh + 4, :])
                    st2a = p3s.tile([128, 4, 6], F32, tag="st2a")
                    mv2a = p3s.tile([128, 4, 2], F32, tag="mv2a")
                    for j in range(4):
                        nc.vector.bn_stats(out=st2a[:, j, :], in_=x2a[:, j, :])
                        nc.vector.bn_aggr(out=mv2a[:, j, :], in_=st2a[:, j, :])
                    sd2a = p3s.tile([128, 4], F32, tag="sd2a")
                    nc.scalar.activation(out=sd2a[:], in_=mv2a[:, :, 1:2],
                                         func=SQRT, bias=s_eps[:])
                    rstd2a = p3s.tile([128, 4], F32, tag="rstd2a")
                    nc.vector.reciprocal(out=rstd2a[:], in_=sd2a[:])
                    xn2a = p3s.tile([128, 4, 128], F32, tag="xn2a")
                    for j in range(4):
                        nc.vector.tensor_scalar(out=xn2a[:, j, :],
                                                in0=x2a[:, j, :],
                                                scalar1=mv2a[:, j, 0:1],
                                                scalar2=rstd2a[:, j:j + 1],
                                                op0=SUB, op1=MUL)
                    g2rep = bass.AP(tensor=s_g2bc.tensor, offset=s_g2bc.offset,
                                    ap=[s_g2bc.ap[0], [0, 4], s_g2bc.ap[1]])
                    b2rep = bass.AP(tensor=s_b2bc.tensor, offset=s_b2bc.offset,
                                    ap=[s_b2bc.ap[0], [0, 4], s_b2bc.ap[1]])
                    ob = p3o.tile([128, 4, 128], F32, tag="ob")
                    tga = p3s.tile([128, 4, 128], F32, tag="tga")
                    nc.vector.tensor_mul(out=tga[:], in0=xn2a[:], in1=g2rep)
                    nc.vector.tensor_add(out=ob[:], in0=tga[:], in1=b2rep)
                    oslice = out.ap()[512 * hh:512 * (hh + 1), :]
                    oslice = oslice.rearrange("(j p) f -> p j f", p=128)
                    nc.sync.dma_start(out=oslice, in_=ob[:])

                # FF half-0 first: its h1T was produced inside the phase-2
                # loop, so its matmuls overlap the half-1 mm3/LN1 DVE chain.
                ff_half(0)

                # mm3 + LN1 + transpose-back for the SECOND 512-node half
                # (the first half ran inside the phase-2 loop)
                psm = ppm.tile([128, 512], F32, tag="psm")
                nc.tensor.matmul(out=psm[:], lhsT=s_w3[:],
                                 rhs=s_aggTb[:, 512:1024],
                                 start=True, stop=True)
                nc.scalar.activation(out=s_a2Tb[:, 512:1024],
                                     in_=psm[:], func=IDENT,
                                     bias=s_mb3x48[:])
                pstT = pp3.tile([128, 4, 128], F32, tag="pstT")
                for b in range(4):
                    nc.tensor.matmul(
                        out=pstT[:, b, :],
                        lhsT=s_a2Tb[:, 512 + 128 * b:512 + 128 * (b + 1)],
                        rhs=s_id[:], start=True, stop=True)
                x1a = p3s.tile([128, 4, 128], F32, tag="x1a")
                nc.vector.tensor_add(out=x1a[:], in0=pstT[:],
                                     in1=s_nhl[:, 4:8, :])
                sta = p3s.tile([128, 4, 6], F32, tag="sta")
                mva = p3s.tile([128, 4, 2], F32, tag="mva")
                for b in range(4):
                    nc.vector.bn_stats(out=sta[:, b, :], in_=x1a[:, b, :])
                    nc.vector.bn_aggr(out=mva[:, b, :], in_=sta[:, b, :])
                sda = p3s.tile([128, 4], F32, tag="sda")
                nc.scalar.activation(out=sda[:], in_=mva[:, :, 1:2],
                                     func=SQRT, bias=s_eps[:])
                rstda = p3s.tile([128, 4], F32, tag="rstda")
                nc.vector.reciprocal(out=rstda[:], in_=sda[:])
                xna = p3s.tile([128, 4, 128], BF, tag="xna")
                for b in range(4):
                    nc.vector.tensor_scalar(out=xna[:, b, :],
                                            in0=x1a[:, b, :],
                                            scalar1=mva[:, b, 0:1],
                                            scalar2=rstda[:, b:b + 1],
                                            op0=SUB, op1=MUL)
                g1rep = bass.AP(tensor=s_g1bc.tensor, offset=s_g1bc.offset,
                                ap=[s_g1bc.ap[0], [0, 4], s_g1bc.ap[1]])
                b1rep = bass.AP(tensor=s_b1bc.tensor, offset=s_b1bc.offset,
                                ap=[s_b1bc.ap[0], [0, 4], s_b1bc.ap[1]])
                tb1a = p3s.tile([128, 4, 128], BF, tag="tb1a")
                nc.vector.tensor_mul(out=tb1a[:], in0=xna[:], in1=g1rep)
                nc.vector.tensor_add(out=s_h1rm[:, 4:8, :], in0=tb1a[:],
                                     in1=b1rep)
                psTT = pp3.tile([128, 4, 128], F32, tag="pstT", name="psTT2")
                for b in range(4):
                    nc.tensor.matmul(out=psTT[:, b, :],
                                     lhsT=s_h1rm[:, 4 + b, :],
                                     rhs=s_id[:], start=True, stop=True)
                nc.scalar.activation(out=s_h1T[:, 512:1024],
                                     in_=psTT[:], func=COPY)
                # FF + LN2 for the second half
                ff_half(1)

    nc.compile()
    return nc


def prep_core_inputs(inputs, n_glob, n_loc, core):
    """Host-side layout prep for one core: slicing, k-major reorder within
    chunks, transposes, dtype casts, tiny constant broadcasts. No kernel math
    (no indexing of data tensors by edge_idx) is done on the host."""
    f32 = np.float32
    n0 = core * n_loc
    E = n_loc * KK
    n_ch = E // CH_E

    def kmajor(x):
        # x: [n_loc, KK, ...] -> per 64-node chunk: [KK, 64, ...] -> flat E
        tail = x.shape[2:]
        x = x.reshape(n_ch, CH_NODES, KK, *tail)
        x = x.transpose(0, 2, 1, *range(3, 3 + len(tail)))
        return np.ascontiguousarray(x.reshape(E, *tail))

    eh = kmajor(inputs["edge_h"][n0:n0 + n_loc])          # [E, H] k-major
    eh = np.ascontiguousarray(eh.T).astype(FP8)           # [H, E] fp8
    j = (inputs["edge_idx"][n0:n0 + n_loc].astype(np.int64) - n0) % n_glob
    j = kmajor(j)                                         # [E] k-major
    idx16 = np.tile(np.ascontiguousarray(j.reshape(E // 16, 16).T), (8, 1)
                    ).astype(np.int16)
    m = kmajor(inputs["ar_mask"][n0:n0 + n_loc])          # [E] k-major
    m8 = np.ascontiguousarray(m.reshape(E // 128, 128).T).astype(FP8)
    one8 = np.float32(1.0).astype(FP8)
    vpk = ((m8.view(np.uint8).astype(np.uint16) << 8)
           | np.uint16(one8.view(np.uint8))).view(np.int16)
    node_hT = np.ascontiguousarray(
        np.roll(inputs["node_h"], -n0, axis=0).T).astype(BF16)
    seqT = np.ascontiguousarray(
        np.roll(inputs["seq_emb"], -n0, axis=0).T).astype(BF16)
    nhl = np.ascontiguousarray(
        inputs["node_h"][n0:n0 + n_loc].reshape(n_loc // 128, 128, H)
        .transpose(1, 0, 2)).astype(f32)
    mW1 = inputs["mW1"]
    bb = np.zeros((128, 2176), dtype=BF16)
    bb[:, 0:128] = mW1[0:128].astype(BF16)
    bb[:, 128:256] = mW1[128:256].astype(BF16)
    bb[:, 256:384] = mW1[384:512].astype(BF16)
    bb[:, 384:512] = mW1[256:384].astype(BF16)
    bb[:, 512:640] = inputs["mW2"].astype(BF16)
    bb[:, 640:768] = inputs["mW3"].astype(BF16)
    bb[:, 768:1280] = inputs["fW1"].astype(BF16)
    bb[:, 1280:1792] = np.ascontiguousarray(
        inputs["fW2"].reshape(4, 128, H).transpose(1, 0, 2)).reshape(
        128, 512).astype(BF16)
    bb[:, 1792:1920] = np.eye(128, dtype=BF16)
    bb[:, 1920:2048] = np.tile(inputs["g1"][None, :], (128, 1)).astype(BF16)
    bb[:, 2048:2176] = np.tile(inputs["b1"][None, :], (128, 1)).astype(BF16)
    bf = np.zeros((128, 1288), dtype=f32)
    bf[:, 0] = inputs["mb1"].astype(f32)
    bf[:, 1] = inputs["mb2"].astype(f32)
    bf[:, 2] = (inputs["mb3"] * KK).astype(f32)
    bf[:, 3:7] = np.ascontiguousarray(inputs["fb1"].reshape(4, 128).T)
    bf[:, 7] = inputs["fb2"].astype(f32)
    bf[:, 8:136] = np.tile(inputs["g2"][None, :], (128, 1))
    bf[:, 136:264] = np.tile(inputs["b2"][None, :], (128, 1))
    bf[:, 264:1288] = nhl.reshape(128, 1024)
    d = {
        "edge_hT": eh, "idx16": idx16,
        "node_hT": node_hT, "seqT": seqT,
        "blob_bf": bb, "blob_f32": bf,
        "id01": np.eye(128, dtype=np.int16), "vpk": vpk,
    }
    return d


_NC_CACHE = {}


def kernel(**inputs):
    inputs = {k: np.asarray(v) for k, v in inputs.items()}
    n_glob = inputs["node_h"].shape[0]
    n_loc = n_glob // N_CORES
    key = (n_glob, n_loc)
    if key not in _NC_CACHE:
        _NC_CACHE[key] = build_nc(n_glob, n_loc, N_CORES)
    nc = _NC_CACHE[key]
    in_maps = [prep_core_inputs(inputs, n_glob, n_loc, c)
               for c in range(N_CORES)]
    res = bass_utils.run_bass_kernel_spmd(nc, in_maps,
                                          core_ids=list(range(N_CORES)))
    return np.concatenate([res.results[c]["out"] for c in range(N_CORES)],
                          axis=0).astype(np.float32)


# revision 35
# speedup vs baseline: 1.1949x; 1.0111x over previous
"""Trainium2 Bass kernel for nn_DecoderLayer (GNN message passing decoder layer).

Math (per reference):
  seq_j = seq_emb[edge_idx] * ar_mask[..., None]
  x = concat([h_i, h_j, edge_h, seq_j], -1)            # [res,k,4h]
  msg = gelu(x @ mW1 + mb1); msg = gelu(msg @ mW2 + mb2); msg = msg @ mW3 + mb3
  agg = msg.sum(1)
  h = LN(node_h + agg) * g1 + b1
  ff = gelu(h @ fW1 + fb1) @ fW2 + fb2
  h = LN(h + ff) * g2 + b2

Strategy (8-way data parallel over the residue dim, no collectives):
  - mm1 decomposed: x@mW1 = h_i@Wa + h_j@Wb + edge_h@Wc + seq_j@Wd.
    h_j@Wb and seq_emb@Wd are precomputed per global node into a fused FP8
    gather table [8192, 256] in DRAM; per-edge rows fetched with dma_gather
    (256B packets - half the DMA-engine packet cost of bf16).
  - edge_h streamed in fp8 as well (mixed fp8xbf16 matmuls keep the weights
    in bf16, so only per-edge data pays fp8 quantization; predicted rel err
    ~5e-3 vs the 2e-2 gate).
  - dma_gather descriptor generation runs on the GpSimd Q7 core-pair selected
    by queue_num; queues 1-3 run concurrently on three disjoint pairs.
  - Edges are k-major within 3072-edge chunks (64 nodes x 48 k): col=k*64+n.
    Aligns 512-col matmul slices with the per-node h_i@Wa broadcast AP and
    makes the k-reduction a log-tree of dense bf16 adds on DVE.
  - ar_mask folded into the PE transpose of the gathered seq half via
    rhs = diag(mask); the diag tiles are built ON DEVICE per chunk with one
    stride-0 int16 DVE multiply (select) from a 96KB packed mask input --
    replaces the 12.6MB host-built diag tensor of the earlier version.
  - k-reduction before mm3 (linearity): 48x less mm3 work.
  - mm2 (+gelu2+reduce) of chunk N is issued interleaved into chunk N+1's
    mm1 stream so ACT always has ready work and PE never waits on gelu1.
"""

import os
import sys

sys.path.insert(0, "/opt/trn_rl_repo")

import numpy as np
import ml_dtypes

import concourse.bacc as bacc
import concourse.bass as bass
import concourse.mybir as mybir
import concourse.tile as tile
from concourse import bass_utils

BF16 = ml_dtypes.bfloat16
FP8 = ml_dtypes.float8_e4m3
F32 = mybir.dt.float32
BF = mybir.dt.bfloat16
I16 = mybir.dt.int16
F8 = mybir.dt.float8e4

RES, KK, H = 8192, 48, 128
N_CORES = 8
CH_NODES = 64                 # nodes per chunk
CH_E = CH_NODES * KK          # 3072 edges per chunk
HC_E = CH_E // 2              # 1536 edges per half-chunk
N_SUB = CH_E // 128           # 24 subtiles of 128 edges per chunk


def build_nc(n_glob, n_loc, num_devices):
    E = n_loc * KK
    n_ch = E // CH_E           # 16 chunks
    nblk = n_loc // 128        # 8 local node blocks
    gblk = n_glob // 128       # 64 global node blocks

    nc = bacc.Bacc("TRN2", target_bir_lowering=False, debug=False,
                   num_devices=num_devices, num_swdge_queues=4)

    def din(name, shape, dt):
        return nc.dram_tensor(name, shape, dt, kind="ExternalInput")

    edge_hT = din("edge_hT", [H, E], F8)            # k-major per chunk, fp8
    idx16 = din("idx16", [128, E // 16], I16)       # k-major per chunk
    node_hT = din("node_hT", [H, n_glob], BF)       # rotated: local first
    seqT = din("seqT", [H, n_glob], BF)
    blob_bf = din("blob_bf", [128, 2176], BF)       # packed bf16 constants
    blob_f32 = din("blob_f32", [128, 1288], F32)    # packed f32 constants
    id01 = din("id01", [128, 128], I16)             # identity as int 0/1
    # vpk[p, s] = (fp8bits(mask) << 8) | fp8bits(1.0): one u16 per edge whose
    # int16-select against id01 yields the interleaved (1.0, m) fp8 pair diag
    # consumed as the DoubleRow rhs.
    vpk = din("vpk", [128, E // 128], I16)
    out = nc.dram_tensor("out", [n_loc, H], F32, kind="ExternalOutput")

    GELU = mybir.ActivationFunctionType.Gelu
    IDENT = mybir.ActivationFunctionType.Identity
    COPY = mybir.ActivationFunctionType.Copy
    SQRT = mybir.ActivationFunctionType.Sqrt
    SUB = mybir.AluOpType.subtract
    MUL = mybir.AluOpType.mult

    with tile.TileContext(nc) as tc:
        with tc.tile_pool(name="singles", bufs=1) as sg, \
             tc.tile_pool(name="dram", bufs=1, space="DRAM") as dp:
            # ---- resident constants: two packed blobs + idx + mask ----
            s_bb = sg.tile([128, 2176], BF)
            # only the Wb/Wd slice now (64KB): it plus the first node/seq
            # quarter are all the table build needs, so the big blob, idx,
            # and f32 constants are deferred behind them in queue order.
            nc.sync.dma_start(out=s_bb[:, 128:384],
                              in_=blob_bf.ap()[:, 128:384])
            s_bf = sg.tile([128, 1288], F32)
            s_idx = sg.tile([128, E // 16], I16)
            s_id01 = sg.tile([128, 128], I16)
            s_vpk = sg.tile([128, E // 128], I16)
            s_wa = s_bb[:, 0:128]
            s_wb = s_bb[:, 128:256]
            s_wd = s_bb[:, 256:384]
            s_wc = s_bb[:, 384:512]
            s_w2 = s_bb[:, 512:640]
            s_w3 = s_bb[:, 640:768]
            s_fw1 = s_bb[:, 768:1280]
            s_fw2 = s_bb[:, 1280:1792].rearrange("p (a b) -> p a b", a=4)
            s_id = s_bb[:, 1792:1920]
            s_g1bc = s_bb[:, 1920:2048]
            s_b1bc = s_bb[:, 2048:2176]
            s_mb1c = s_bf[:, 0:1]
            s_mb2c = s_bf[:, 1:2]
            s_mb3x48 = s_bf[:, 2:3]
            s_fb1c = s_bf[:, 3:7]
            s_fb2c = s_bf[:, 7:8]
            s_g2bc = s_bf[:, 8:136]
            s_b2bc = s_bf[:, 136:264]
            s_nhl = s_bf[:, 264:1288].rearrange("p (a b) -> p a b", a=nblk)
            s_eps = sg.tile([128, 1], F32)
            nc.vector.memset(s_eps[:], 1e-5)

            s_aT = sg.tile([128, n_loc], BF)        # (Wa^T h_i) per local node
            s_aggTb = sg.tile([128, n_loc], BF)     # k-sum of msg2, fm bf16
            s_a2Tb = sg.tile([128, n_loc], BF)
            s_h1T = sg.tile([128, n_loc], BF)
            s_h1rm = sg.tile([128, nblk, H], BF)

            table = dp.tile([n_glob, 256], F8)

            # ---- phase 1: gather table (fp8) + Wa precompute ----
            # (phase-2 SBUF pools open first so their zone sits below the
            # phase-1 pools on the allocator stack: phase-2 DMAs can then
            # start during phase 1 instead of waiting for its release)
            ctx2 = [tc.tile_pool(name="p2g", bufs=6),
                    tc.tile_pool(name="p2e", bufs=3),
                    tc.tile_pool(name="p2d", bufs=3),
                    tc.tile_pool(name="p2t2", bufs=4),
                    tc.tile_pool(name="p2r", bufs=2),
                    tc.tile_pool(name="p2t4", bufs=3),
                    tc.tile_pool(name="p2x", bufs=1)]
            p2g, p2e, p2d, p2t2, p2r, p2t4, p2x = [c.__enter__() for c in ctx2]
            with tc.tile_pool(name="p1s", bufs=1) as p1s, \
                 tc.tile_pool(name="p1p", bufs=2, space="PSUM") as p1p, \
                 tc.tile_pool(name="p1p2", bufs=3, space="PSUM") as p1p2:
                nhT_bf = p1s.tile([128, n_glob], BF, tag="big1")
                seT_bf = p1s.tile([128, n_glob], BF, tag="big2")
                qn = n_glob // 4
                nc.sync.dma_start(out=nhT_bf[:, 0:qn],
                                  in_=node_hT.ap()[:, 0:qn])
                nc.scalar.dma_start(out=seT_bf[:, 0:qn],
                                    in_=seqT.ap()[:, 0:qn])
                nc.sync.dma_start(out=s_idx[:], in_=idx16.ap())
                nc.sync.dma_start(out=s_bb[:, 0:128],
                                  in_=blob_bf.ap()[:, 0:128])
                nc.sync.dma_start(out=s_bb[:, 384:2176],
                                  in_=blob_bf.ap()[:, 384:2176])
                nc.scalar.dma_start(out=s_bf[:], in_=blob_f32.ap())
                nc.scalar.dma_start(out=s_id01[:], in_=id01.ap())
                nc.scalar.dma_start(out=s_vpk[:], in_=vpk.ap())
                for qq in range(1, 4):
                    nc.sync.dma_start(out=nhT_bf[:, qn * qq:qn * (qq + 1)],
                                      in_=node_hT.ap()[:, qn * qq:qn * (qq + 1)])
                    nc.scalar.dma_start(out=seT_bf[:, qn * qq:qn * (qq + 1)],
                                        in_=seqT.ap()[:, qn * qq:qn * (qq + 1)])
                # table rows: node-major fp8, built 4 blocks (512 nodes) per
                # PSUM tile so the copy/write pipeline amortizes hop latency.
                tstage = p1s.tile([128, gblk, 256], F8, tag="tstage")
                for gq in range(gblk // 4):
                    ps4 = p1p2.tile([128, 4, 256], F32, tag="tps4")
                    for j in range(4):
                        b = 4 * gq + j
                        nc.tensor.matmul(out=ps4[:, j, 0:128],
                                         lhsT=nhT_bf[:, 128 * b:128 * (b + 1)],
                                         rhs=s_wb[:], start=True, stop=True)
                        nc.tensor.matmul(out=ps4[:, j, 128:256],
                                         lhsT=seT_bf[:, 128 * b:128 * (b + 1)],
                                         rhs=s_wd[:], start=True, stop=True)
                    if gq % 2 == 0:
                        nc.scalar.activation(out=tstage[:, 4 * gq:4 * gq + 4, :],
                                             in_=ps4[:], func=COPY)
                    else:
                        nc.vector.tensor_copy(out=tstage[:, 4 * gq:4 * gq + 4, :],
                                              in_=ps4[:])
                    if gq % 2 == 1:
                        g0 = 4 * (gq - 1)
                        tslice = table[128 * g0:128 * (g0 + 8), :]
                        tslice = tslice.rearrange("(b p) f -> p b f", p=128)
                        nc.sync.dma_start(out=tslice,
                                          in_=tstage[:, g0:g0 + 8, :])
                # aT = Wa^T h for local nodes (after the table: the table
                # gates the gathers, aT only gates chunk-0 mm1)
                for hh in range(n_loc // 512):
                    psa = p1p.tile([128, 512], F32, tag="psa")
                    nc.tensor.matmul(out=psa[:], lhsT=s_wa[:],
                                     rhs=nhT_bf[:, 512 * hh:512 * (hh + 1)],
                                     start=True, stop=True)
                    nc.scalar.activation(out=s_aT[:, 512 * hh:512 * (hh + 1)],
                                         in_=psa[:], func=COPY)

            # ---- phase 2: main edge loop, k-major chunks ----
            with tc.tile_pool(name="pp1", bufs=2, space="PSUM") as pp1, \
                 tc.tile_pool(name="ppw", bufs=2, space="PSUM") as ppw:
                def w2_stage(pend, hc):
                    t2s, t4p = pend
                    e0 = HC_E * hc
                    for b in range(3):
                        psw = ppw.tile([128, 512], F32, tag="psw",
                                       name=f"psw{hc}{b}")
                        nc.tensor.matmul(out=psw[:], lhsT=s_w2[:],
                                         rhs=t2s[hc][:, 512 * b:512 * (b + 1)],
                                         start=True, stop=True)
                        nc.scalar.activation(
                            out=t4p[:, e0 + 512 * b:e0 + 512 * (b + 1)],
                            in_=psw[:], func=GELU, bias=s_mb2c[:])

                def reduce_stage(pendx, _unused, chp):
                    # dense log-tree over the k-major layout (DVE; GpSimd's
                    # strict FIFO is owned by the gather instructions)
                    _, t4p = pendx[0]
                    r1 = p2r.tile([128, HC_E], BF, tag="r1")
                    nc.vector.tensor_add(out=r1[:], in0=t4p[:, 0:HC_E],
                                         in1=t4p[:, HC_E:CH_E])
                    nc.vector.tensor_add(out=r1[:, 0:768], in0=r1[:, 0:768],
                                         in1=r1[:, 768:1536])
                    nc.vector.tensor_add(out=r1[:, 0:384], in0=r1[:, 0:384],
                                         in1=r1[:, 384:768])
                    nc.vector.tensor_add(out=r1[:, 0:192], in0=r1[:, 0:192],
                                         in1=r1[:, 192:384])
                    nc.vector.tensor_add(out=r1[:, 0:64], in0=r1[:, 0:64],
                                         in1=r1[:, 64:128])
                    nc.vector.tensor_add(out=s_aggTb[:, CH_NODES * chp:
                                                     CH_NODES * (chp + 1)],
                                         in0=r1[:, 0:64], in1=r1[:, 128:192])

                pend = None
                for ch in range(n_ch):
                    # each chunk's gather split across swdge queues so
                    # descriptor generation runs in parallel; the first two
                    # chunks split 4 ways to cut the pipeline-fill latency
                    g = p2g.tile([128, N_SUB, 256], F8, tag="g")
                    nsplit = 4 if ch < 2 else 2
                    sub_s = N_SUB // nsplit
                    idx_s = (CH_E // 16) // nsplit
                    for hg in range(nsplit):
                        nc.gpsimd.dma_gather(
                            out_ap=g[:, sub_s * hg:sub_s * (hg + 1), :],
                            in_ap=table[:],
                            idxs_ap=s_idx[:, (CH_E // 16) * ch + idx_s * hg:
                                          (CH_E // 16) * ch + idx_s * (hg + 1)],
                            num_idxs=CH_E // nsplit,
                            num_idxs_reg=CH_E // nsplit,
                            elem_size=256,
                            single_packet=False,
                            queue_num=(2 * ch + hg) % 4,
                        )
                    e = p2e.tile([128, CH_E], F8, tag="e")
                    nc.sync.dma_start(out=e[:],
                                      in_=edge_hT.ap()[:, CH_E * ch:
                                                       CH_E * (ch + 1)])
                    # diag for this chunk (int16 select on DVE; GpSimd/Pool
                    # rejects int16 mult):
                    # dia_u16[p, s, n] = id01[p, n] * vpk[p, 24*ch + s];
                    # bitcast as fp8 pairs it is diag((1.0, m)) per subtile.
                    dia = p2d.tile([128, N_SUB, 128], I16, tag="dia")
                    idb = bass.AP(tensor=s_id01.tensor, offset=s_id01.offset,
                                  ap=[s_id01.ap[0], [0, N_SUB], s_id01.ap[1]])
                    vsl = s_vpk[:, N_SUB * ch:N_SUB * (ch + 1)]
                    vb = bass.AP(tensor=vsl.tensor, offset=vsl.offset,
                                 ap=[vsl.ap[0], vsl.ap[1], [0, 128]])
                    nc.vector.tensor_mul(out=dia[:], in0=idb, in1=vb)
                    dg8 = dia[:].bitcast(F8)

                    t4 = p2t4.tile([128, CH_E], BF, tag="t4")
                    na = s_aT[:, CH_NODES * ch:CH_NODES * (ch + 1)]
                    rep = bass.AP(tensor=na.tensor, offset=na.offset,
                                  ap=[na.ap[0], [0, 8], na.ap[1]])
                    t2s = []
                    for hc in range(2):
                        ps1 = pp1.tile([128, 3, 512], F32, tag="ps1")
                        e0 = HC_E * hc  # edge col offset within chunk
                        for b in range(3):
                            nc.tensor.matmul(
                                out=ps1[:, b, :], lhsT=s_wc[:],
                                rhs=e[:, e0 + 512 * b:e0 + 512 * (b + 1)],
                                start=True, stop=False)
                        for b in range(3):
                            nc.tensor.matmul(out=ps1[:, b, :], lhsT=s_id[:],
                                             rhs=rep, start=False, stop=False)
                        for sub in range(12):
                            gsub = 12 * hc + sub
                            bank = sub // 4
                            col = 128 * (sub % 4)
                            # fused DoubleRow: psum += h_j + m * seq_j in one
                            # matmul (planar lhsT pairs, interleaved rhs pairs)
                            gs = g[:, gsub, :]
                            lhsT = bass.AP(tensor=gs.tensor, offset=gs.offset,
                                           ap=[gs.ap[0], [128, 2], [1, 128]])
                            ds = dg8[:, gsub, :]
                            rhsd = bass.AP(tensor=ds.tensor, offset=ds.offset,
                                           ap=[ds.ap[0], [1, 2], [2, 128]])
                            nc.tensor.matmul(
                                out=ps1[:, bank, col:col + 128],
                                lhsT=lhsT, rhs=rhsd,
                                start=False, stop=True,
                                perf_mode=mybir.MatmulPerfMode.DoubleRow)
                        # previous chunk's w2 stage first: its gelu2 inputs
                        # are ready, so ACT drains them while PE works here.
                        if pend is not None:
                            w2_stage(pend[0], hc)
                            if hc == 1:
                                reduce_stage((pend[0],), None, pend[1])
                        t2 = p2t2.tile([128, HC_E], BF, tag="t2")
                        nc.scalar.activation(out=t2[:], in_=ps1[:], func=GELU,
                                             bias=s_mb1c[:])
                        t2s.append(t2)
                    pend = ((t2s, t4), ch)
                    # phase-3 head start for the first 512 nodes (chunks 0-7
                    # fully reduced by iteration 9): mm3, LN1, and the
                    # transpose back ride spare ppw psum slots mid-loop.
                    if ch == 10:
                        psm0 = ppw.tile([128, 512], F32, tag="psw",
                                        name="mm3h0")
                        nc.tensor.matmul(out=psm0[:], lhsT=s_w3[:],
                                         rhs=s_aggTb[:, 0:512],
                                         start=True, stop=True)
                        nc.scalar.activation(out=s_a2Tb[:, 0:512],
                                             in_=psm0[:], func=IDENT,
                                             bias=s_mb3x48[:])
                    if ch == 11:
                        psT0 = ppw.tile([128, 512], F32, tag="psw",
                                        name="lnT0")
                        v0 = psT0[:].rearrange("p (a b) -> p a b", a=4)
                        for b in range(4):
                            nc.tensor.matmul(
                                out=v0[:, b, :],
                                lhsT=s_a2Tb[:, 128 * b:128 * (b + 1)],
                                rhs=s_id[:], start=True, stop=True)
                        x1h = p2x.tile([128, 4, 128], F32, tag="x1h")
                        nc.vector.tensor_add(out=x1h[:], in0=v0,
                                             in1=s_nhl[:, 0:4, :])
                        sth = p2x.tile([128, 4, 6], F32, tag="sth")
                        mvh = p2x.tile([128, 4, 2], F32, tag="mvh")
                        for b in range(4):
                            nc.vector.bn_stats(out=sth[:, b, :],
                                               in_=x1h[:, b, :])
                            nc.vector.bn_aggr(out=mvh[:, b, :],
                                              in_=sth[:, b, :])
                        sdh = p2x.tile([128, 4], F32, tag="sdh")
                        nc.scalar.activation(out=sdh[:], in_=mvh[:, :, 1:2],
                                             func=SQRT, bias=s_eps[:])
                        rsh = p2x.tile([128, 4], F32, tag="rsh")
                        nc.vector.reciprocal(out=rsh[:], in_=sdh[:])
                        xnh = p2x.tile([128, 4, 128], BF, tag="xnh")
                        for b in range(4):
                            nc.vector.tensor_scalar(out=xnh[:, b, :],
                                                    in0=x1h[:, b, :],
                                                    scalar1=mvh[:, b, 0:1],
                                                    scalar2=rsh[:, b:b + 1],
                                                    op0=SUB, op1=MUL)
                        g1r4 = bass.AP(tensor=s_g1bc.tensor,
                                       offset=s_g1bc.offset,
                                       ap=[s_g1bc.ap[0], [0, 4], s_g1bc.ap[1]])
                        b1r4 = bass.AP(tensor=s_b1bc.tensor,
                                       offset=s_b1bc.offset,
                                       ap=[s_b1bc.ap[0], [0, 4], s_b1bc.ap[1]])
                        tbh = p2x.tile([128, 4, 128], BF, tag="tbh")
                        nc.vector.tensor_mul(out=tbh[:], in0=xnh[:], in1=g1r4)
                        nc.vector.tensor_add(out=s_h1rm[:, 0:4, :],
                                             in0=tbh[:], in1=b1r4)
                    if ch == 12:
                        psT1 = ppw.tile([128, 512], F32, tag="psw",
                                        name="h1T0")
                        v1 = psT1[:].rearrange("p (a b) -> p a b", a=4)
                        for b in range(4):
                            nc.tensor.matmul(out=v1[:, b, :],
                                             lhsT=s_h1rm[:, b, :],
                                             rhs=s_id[:], start=True,
                                             stop=True)
                        nc.scalar.activation(out=s_h1T[:, 0:512],
                                             in_=psT1[:], func=COPY)
                w2_stage(pend[0], 0)
                w2_stage(pend[0], 1)
                reduce_stage((pend[0],), None, pend[1])

            for c in reversed(ctx2):
                c.__exit__(None, None, None)

            # ---- phase 3: mm3, LN1, FF, LN2, output (stage-batched) ----
            with tc.tile_pool(name="p3s", bufs=8) as p3s, \
                 tc.tile_pool(name="p3u", bufs=4) as p3u, \
                 tc.tile_pool(name="p3o", bufs=2) as p3o, \
                 tc.tile_pool(name="pp3", bufs=1, space="PSUM") as pp3, \
                 tc.tile_pool(name="ppm", bufs=1, space="PSUM") as ppm, \
                 tc.tile_pool(name="ppf", bufs=2, space="PSUM") as ppf, \
                 tc.tile_pool(name="ppf2", bufs=2, space="PSUM") as ppf2:
                nh_half = n_loc // 512
                # mm3 + LN1 + transpose-back for the SECOND 512-node half
                # (the first half ran inside the phase-2 loop)
                psm = ppm.tile([128, 512], F32, tag="psm")
                nc.tensor.matmul(out=psm[:], lhsT=s_w3[:],
                                 rhs=s_aggTb[:, 512:1024],
                                 start=True, stop=True)
                nc.scalar.activation(out=s_a2Tb[:, 512:1024],
                                     in_=psm[:], func=IDENT,
                                     bias=s_mb3x48[:])
                pstT = pp3.tile([128, 4, 128], F32, tag="pstT")
                for b in range(4):
                    nc.tensor.matmul(
                        out=pstT[:, b, :],
                        lhsT=s_a2Tb[:, 512 + 128 * b:512 + 128 * (b + 1)],
                        rhs=s_id[:], start=True, stop=True)
                x1a = p3s.tile([128, 4, 128], F32, tag="x1a")
                nc.vector.tensor_add(out=x1a[:], in0=pstT[:],
                                     in1=s_nhl[:, 4:8, :])
                sta = p3s.tile([128, 4, 6], F32, tag="sta")
                mva = p3s.tile([128, 4, 2], F32, tag="mva")
                for b in range(4):
                    nc.vector.bn_stats(out=sta[:, b, :], in_=x1a[:, b, :])
                    nc.vector.bn_aggr(out=mva[:, b, :], in_=sta[:, b, :])
                sda = p3s.tile([128, 4], F32, tag="sda")
                nc.scalar.activation(out=sda[:], in_=mva[:, :, 1:2],
                                     func=SQRT, bias=s_eps[:])
                rstda = p3s.tile([128, 4], F32, tag="rstda")
                nc.vector.reciprocal(out=rstda[:], in_=sda[:])
                xna = p3s.tile([128, 4, 128], BF, tag="xna")
                for b in range(4):
                    nc.vector.tensor_scalar(out=xna[:, b, :],
                                            in0=x1a[:, b, :],
                                            scalar1=mva[:, b, 0:1],
                                            scalar2=rstda[:, b:b + 1],
                                            op0=SUB, op1=MUL)
                g1rep = bass.AP(tensor=s_g1bc.tensor, offset=s_g1bc.offset,
                                ap=[s_g1bc.ap[0], [0, 4], s_g1bc.ap[1]])
                b1rep = bass.AP(tensor=s_b1bc.tensor, offset=s_b1bc.offset,
                                ap=[s_b1bc.ap[0], [0, 4], s_b1bc.ap[1]])
                tb1a = p3s.tile([128, 4, 128], BF, tag="tb1a")
                nc.vector.tensor_mul(out=tb1a[:], in0=xna[:], in1=g1rep)
                nc.vector.tensor_add(out=s_h1rm[:, 4:8, :], in0=tb1a[:],
                                     in1=b1rep)
                psTT = pp3.tile([128, 4, 128], F32, tag="pstT", name="psTT2")
                for b in range(4):
                    nc.tensor.matmul(out=psTT[:, b, :],
                                     lhsT=s_h1rm[:, 4 + b, :],
                                     rhs=s_id[:], start=True, stop=True)
                nc.scalar.activation(out=s_h1T[:, 512:1024],
                                     in_=psTT[:], func=COPY)
                # FF + LN2 per 512-node half
                for hh in range(nh_half):
                    us = []
                    for fc in range(4):
                        psf = ppf.tile([128, 512], F32, tag="psf")
                        nc.tensor.matmul(out=psf[:],
                                         lhsT=s_fw1[:, 128 * fc:128 * (fc + 1)],
                                         rhs=s_h1T[:, 512 * hh:512 * (hh + 1)],
                                         start=True, stop=True)
                        u = p3u.tile([128, 512], BF, tag=f"u{fc}")
                        nc.scalar.activation(out=u[:], in_=psf[:], func=GELU,
                                             bias=s_fb1c[:, fc:fc + 1])
                        us.append(u)
                    psf2 = ppf2.tile([128, 512], F32, tag="psf2")
                    for fc in range(4):
                        nc.tensor.matmul(out=psf2[:], lhsT=s_fw2[:, fc, :],
                                         rhs=us[fc][:], start=(fc == 0),
                                         stop=(fc == 3))
                    ffT4 = p3s.tile([128, 4, 128], BF, tag="ffT4")
                    nc.scalar.activation(out=ffT4[:], in_=psf2[:],
                                         func=IDENT, bias=s_fb2c[:])
                    psr2T = pp3.tile([128, 4, 128], F32, tag="psr2T")
                    for j in range(4):
                        nc.tensor.matmul(out=psr2T[:, j, :],
                                         lhsT=ffT4[:, j, :],
                                         rhs=s_id[:], start=True, stop=True)
                    x2a = p3s.tile([128, 4, 128], F32, tag="x2a")
                    nc.vector.tensor_add(out=x2a[:], in0=psr2T[:],
                                         in1=s_h1rm[:, 4 * hh:4 * hh + 4, :])
                    st2a = p3s.tile([128, 4, 6], F32, tag="st2a")
                    mv2a = p3s.tile([128, 4, 2], F32, tag="mv2a")
                    for j in range(4):
                        nc.vector.bn_stats(out=st2a[:, j, :], in_=x2a[:, j, :])
                        nc.vector.bn_aggr(out=mv2a[:, j, :], in_=st2a[:, j, :])
                    sd2a = p3s.tile([128, 4], F32, tag="sd2a")
                    nc.scalar.activation(out=sd2a[:], in_=mv2a[:, :, 1:2],
                                         func=SQRT, bias=s_eps[:])
                    rstd2a = p3s.tile([128, 4], F32, tag="rstd2a")
                    nc.vector.reciprocal(out=rstd2a[:], in_=sd2a[:])
                    xn2a = p3s.tile([128, 4, 128], F32, tag="xn2a")
                    for j in range(4):
                        nc.vector.tensor_scalar(out=xn2a[:, j, :],
                                                in0=x2a[:, j, :],
                                                scalar1=mv2a[:, j, 0:1],
                                                scalar2=rstd2a[:, j:j + 1],
                                                op0=SUB, op1=MUL)
                    g2rep = bass.AP(tensor=s_g2bc.tensor, offset=s_g2bc.offset,
                                    ap=[s_g2bc.ap[0], [0, 4], s_g2bc.ap[1]])
                    b2rep = bass.AP(tensor=s_b2bc.tensor, offset=s_b2bc.offset,
                                    ap=[s_b2bc.ap[0], [0, 4], s_b2bc.ap[1]])
                    ob = p3o.tile([128, 4, 128], F32, tag="ob")
                    tga = p3s.tile([128, 4, 128], F32, tag="tga")
                    nc.vector.tensor_mul(out=tga[:], in0=xn2a[:], in1=g2rep)
                    nc.vector.tensor_add(out=ob[:], in0=tga[:], in1=b2rep)
                    oslice = out.ap()[512 * hh:512 * (hh + 1), :]
                    oslice = oslice.rearrange("(j p) f -> p j f", p=128)
                    nc.sync.dma_start(out=oslice, in_=ob[:])

    nc.compile()
    return nc


def prep_core_inputs(inputs, n_glob, n_loc, core):
    """Host-side layout prep for one core: slicing, k-major reorder within
    chunks, transposes, dtype casts, tiny constant broadcasts. No kernel math
    (no indexing of data tensors by edge_idx) is done on the host."""
    f32 = np.float32
    n0 = core * n_loc
    E = n_loc * KK
    n_ch = E // CH_E

    def kmajor(x):
        # x: [n_loc, KK, ...] -> per 64-node chunk: [KK, 64, ...] -> flat E
        tail = x.shape[2:]
        x = x.reshape(n_ch, CH_NODES, KK, *tail)
        x = x.transpose(0, 2, 1, *range(3, 3 + len(tail)))
        return np.ascontiguousarray(x.reshape(E, *tail))

    eh = kmajor(inputs["edge_h"][n0:n0 + n_loc])          # [E, H] k-major
    eh = np.ascontiguousarray(eh.T).astype(FP8)           # [H, E] fp8
    j = (inputs["edge_idx"][n0:n0 + n_loc].astype(np.int64) - n0) % n_glob
    j = kmajor(j)                                         # [E] k-major
    idx16 = np.tile(np.ascontiguousarray(j.reshape(E // 16, 16).T), (8, 1)
                    ).astype(np.int16)
    m = kmajor(inputs["ar_mask"][n0:n0 + n_loc])          # [E] k-major
    m8 = np.ascontiguousarray(m.reshape(E // 128, 128).T).astype(FP8)
    one8 = np.float32(1.0).astype(FP8)
    vpk = ((m8.view(np.uint8).astype(np.uint16) << 8)
           | np.uint16(one8.view(np.uint8))).view(np.int16)
    node_hT = np.ascontiguousarray(
        np.roll(inputs["node_h"], -n0, axis=0).T).astype(BF16)
    seqT = np.ascontiguousarray(
        np.roll(inputs["seq_emb"], -n0, axis=0).T).astype(BF16)
    nhl = np.ascontiguousarray(
        inputs["node_h"][n0:n0 + n_loc].reshape(n_loc // 128, 128, H)
        .transpose(1, 0, 2)).astype(f32)
    mW1 = inputs["mW1"]
    bb = np.zeros((128, 2176), dtype=BF16)
    bb[:, 0:128] = mW1[0:128].astype(BF16)
    bb[:, 128:256] = mW1[128:256].astype(BF16)
    bb[:, 256:384] = mW1[384:512].astype(BF16)
    bb[:, 384:512] = mW1[256:384].astype(BF16)
    bb[:, 512:640] = inputs["mW2"].astype(BF16)
    bb[:, 640:768] = inputs["mW3"].astype(BF16)
    bb[:, 768:1280] = inputs["fW1"].astype(BF16)
    bb[:, 1280:1792] = np.ascontiguousarray(
        inputs["fW2"].reshape(4, 128, H).transpose(1, 0, 2)).reshape(
        128, 512).astype(BF16)
    bb[:, 1792:1920] = np.eye(128, dtype=BF16)
    bb[:, 1920:2048] = np.tile(inputs["g1"][None, :], (128, 1)).astype(BF16)
    bb[:, 2048:2176] = np.tile(inputs["b1"][None, :], (128, 1)).astype(BF16)
    bf = np.zeros((128, 1288), dtype=f32)
    bf[:, 0] = inputs["mb1"].astype(f32)
    bf[:, 1] = inputs["mb2"].astype(f32)
    bf[:, 2] = (inputs["mb3"] * KK).astype(f32)
    bf[:, 3:7] = np.ascontiguousarray(inputs["fb1"].reshape(4, 128).T)
    bf[:, 7] = inputs["fb2"].astype(f32)
    bf[:, 8:136] = np.tile(inputs["g2"][None, :], (128, 1))
    bf[:, 136:264] = np.tile(inputs["b2"][None, :], (128, 1))
    bf[:, 264:1288] = nhl.reshape(128, 1024)
    d = {
        "edge_hT": eh, "idx16": idx16,
        "node_hT": node_hT, "seqT": seqT,
        "blob_bf": bb, "blob_f32": bf,
        "id01": np.eye(128, dtype=np.int16), "vpk": vpk,
    }
    return d


_NC_CACHE = {}


def kernel(**inputs):
    inputs = {k: np.asarray(v) for k, v in inputs.items()}
    n_glob = inputs["node_h"].shape[0]
    n_loc = n_glob // N_CORES
    key = (n_glob, n_loc)
    if key not in _NC_CACHE:
        _NC_CACHE[key] = build_nc(n_glob, n_loc, N_CORES)
    nc = _NC_CACHE[key]
    in_maps = [prep_core_inputs(inputs, n_glob, n_loc, c)
               for c in range(N_CORES)]
    res = bass_utils.run_bass_kernel_spmd(nc, in_maps,
                                          core_ids=list(range(N_CORES)))
    return np.concatenate([res.results[c]["out"] for c in range(N_CORES)],
                          axis=0).astype(np.float32)


# revision 37
# speedup vs baseline: 1.2289x; 1.0285x over previous
"""Trainium2 Bass kernel for nn_DecoderLayer (GNN message passing decoder layer).

Math (per reference):
  seq_j = seq_emb[edge_idx] * ar_mask[..., None]
  x = concat([h_i, h_j, edge_h, seq_j], -1)            # [res,k,4h]
  msg = gelu(x @ mW1 + mb1); msg = gelu(msg @ mW2 + mb2); msg = msg @ mW3 + mb3
  agg = msg.sum(1)
  h = LN(node_h + agg) * g1 + b1
  ff = gelu(h @ fW1 + fb1) @ fW2 + fb2
  h = LN(h + ff) * g2 + b2

Strategy (8-way data parallel over the residue dim, no collectives):
  - mm1 decomposed: x@mW1 = h_i@Wa + h_j@Wb + edge_h@Wc + seq_j@Wd.
    h_j@Wb and seq_emb@Wd are precomputed per global node into a fused FP8
    gather table [8192, 256] in DRAM; per-edge rows fetched with dma_gather
    (256B packets - half the DMA-engine packet cost of bf16).
  - edge_h streamed in fp8 as well (mixed fp8xbf16 matmuls keep the weights
    in bf16, so only per-edge data pays fp8 quantization; predicted rel err
    ~5e-3 vs the 2e-2 gate).
  - dma_gather descriptor generation runs on the GpSimd Q7 core-pair selected
    by queue_num; queues 1-3 run concurrently on three disjoint pairs.
  - Edges are k-major within 3072-edge chunks (64 nodes x 48 k): col=k*64+n.
    Aligns 512-col matmul slices with the per-node h_i@Wa broadcast AP and
    makes the k-reduction a log-tree of dense bf16 adds on DVE.
  - ar_mask folded into the PE transpose of the gathered seq half via
    rhs = diag(mask); the diag tiles are built ON DEVICE per chunk with one
    stride-0 int16 DVE multiply (select) from a 96KB packed mask input --
    replaces the 12.6MB host-built diag tensor of the earlier version.
  - k-reduction before mm3 (linearity): 48x less mm3 work.
  - mm2 (+gelu2+reduce) of chunk N is issued interleaved into chunk N+1's
    mm1 stream so ACT always has ready work and PE never waits on gelu1.
"""

import os
import sys

sys.path.insert(0, "/opt/trn_rl_repo")

import numpy as np
import ml_dtypes

import concourse.bacc as bacc
import concourse.bass as bass
import concourse.mybir as mybir
import concourse.tile as tile
from concourse import bass_utils

BF16 = ml_dtypes.bfloat16
FP8 = ml_dtypes.float8_e4m3
F32 = mybir.dt.float32
BF = mybir.dt.bfloat16
I16 = mybir.dt.int16
F8 = mybir.dt.float8e4

RES, KK, H = 8192, 48, 128
N_CORES = 8
CH_NODES = 64                 # nodes per chunk
CH_E = CH_NODES * KK          # 3072 edges per chunk
HC_E = CH_E // 2              # 1536 edges per half-chunk
N_SUB = CH_E // 128           # 24 subtiles of 128 edges per chunk


def build_nc(n_glob, n_loc, num_devices):
    E = n_loc * KK
    n_ch = E // CH_E           # 16 chunks
    nblk = n_loc // 128        # 8 local node blocks
    gblk = n_glob // 128       # 64 global node blocks

    nc = bacc.Bacc("TRN2", target_bir_lowering=False, debug=False,
                   num_devices=num_devices, num_swdge_queues=4)

    def din(name, shape, dt):
        return nc.dram_tensor(name, shape, dt, kind="ExternalInput")

    edge_hT = din("edge_hT", [H, E], F8)            # k-major per chunk, fp8
    idx16 = din("idx16", [128, E // 16], I16)       # k-major per chunk
    node_hT = din("node_hT", [H, n_glob], BF)       # rotated: local first
    seqT = din("seqT", [H, n_glob], BF)
    blob_bf = din("blob_bf", [128, 2176], BF)       # packed bf16 constants
    blob_f32 = din("blob_f32", [128, 1288], F32)    # packed f32 constants
    id01 = din("id01", [128, 128], I16)             # identity as int 0/1
    # vpk[p, s] = (fp8bits(mask) << 8) | fp8bits(1.0): one u16 per edge whose
    # int16-select against id01 yields the interleaved (1.0, m) fp8 pair diag
    # consumed as the DoubleRow rhs.
    vpk = din("vpk", [128, E // 128], I16)
    out = nc.dram_tensor("out", [n_loc, H], F32, kind="ExternalOutput")

    GELU = mybir.ActivationFunctionType.Gelu
    IDENT = mybir.ActivationFunctionType.Identity
    COPY = mybir.ActivationFunctionType.Copy
    SQRT = mybir.ActivationFunctionType.Sqrt
    SUB = mybir.AluOpType.subtract
    MUL = mybir.AluOpType.mult

    with tile.TileContext(nc) as tc:
        with tc.tile_pool(name="singles", bufs=1) as sg, \
             tc.tile_pool(name="dram", bufs=1, space="DRAM") as dp:
            # ---- resident constants: two packed blobs + idx + mask ----
            s_bb = sg.tile([128, 2176], BF)
            nc.sync.dma_start(out=s_bb[:], in_=blob_bf.ap())
            s_bf = sg.tile([128, 1288], F32)
            nc.scalar.dma_start(out=s_bf[:], in_=blob_f32.ap())
            s_idx = sg.tile([128, E // 16], I16)
            nc.sync.dma_start(out=s_idx[:], in_=idx16.ap())
            s_id01 = sg.tile([128, 128], I16)
            nc.scalar.dma_start(out=s_id01[:], in_=id01.ap())
            s_vpk = sg.tile([128, E // 128], I16)
            nc.scalar.dma_start(out=s_vpk[:], in_=vpk.ap())
            s_wa = s_bb[:, 0:128]
            s_wb = s_bb[:, 128:256]
            s_wd = s_bb[:, 256:384]
            s_wc = s_bb[:, 384:512]
            s_w2 = s_bb[:, 512:640]
            s_w3 = s_bb[:, 640:768]
            s_fw1 = s_bb[:, 768:1280]
            s_fw2 = s_bb[:, 1280:1792].rearrange("p (a b) -> p a b", a=4)
            s_id = s_bb[:, 1792:1920]
            s_g1bc = s_bb[:, 1920:2048]
            s_b1bc = s_bb[:, 2048:2176]
            s_mb1c = s_bf[:, 0:1]
            s_mb2c = s_bf[:, 1:2]
            s_mb3x48 = s_bf[:, 2:3]
            s_fb1c = s_bf[:, 3:7]
            s_fb2c = s_bf[:, 7:8]
            s_g2bc = s_bf[:, 8:136]
            s_b2bc = s_bf[:, 136:264]
            s_nhl = s_bf[:, 264:1288].rearrange("p (a b) -> p a b", a=nblk)
            s_eps = sg.tile([128, 1], F32)
            nc.vector.memset(s_eps[:], 1e-5)

            s_x1h = sg.tile([128, 4, 128], F32)     # hoisted LN1-h0 input
            s_mvh = sg.tile([128, 4, 2], F32)       # hoisted LN1-h0 mean/var
            s_aT = sg.tile([128, n_loc], BF)        # (Wa^T h_i) per local node
            s_aggTb = sg.tile([128, n_loc], BF)     # k-sum of msg2, fm bf16
            s_a2Tb = sg.tile([128, n_loc], BF)
            s_h1T = sg.tile([128, n_loc], BF)
            s_h1rm = sg.tile([128, nblk, H], BF)

            table = dp.tile([n_glob, 256], F8)

            # ---- phase 1: gather table (fp8) + Wa precompute ----
            # (phase-2 SBUF pools open first so their zone sits below the
            # phase-1 pools on the allocator stack: phase-2 DMAs can then
            # start during phase 1 instead of waiting for its release)
            ctx2 = [tc.tile_pool(name="p2g", bufs=6),
                    tc.tile_pool(name="p2e", bufs=3),
                    tc.tile_pool(name="p2d", bufs=3),
                    tc.tile_pool(name="p2t2", bufs=4),
                    tc.tile_pool(name="p2r", bufs=2),
                    tc.tile_pool(name="p2t4", bufs=3),
                    tc.tile_pool(name="p2x", bufs=1)]
            p2g, p2e, p2d, p2t2, p2r, p2t4, p2x = [c.__enter__() for c in ctx2]
            with tc.tile_pool(name="p1s", bufs=1) as p1s, \
                 tc.tile_pool(name="p1p", bufs=2, space="PSUM") as p1p, \
                 tc.tile_pool(name="p1p2", bufs=3, space="PSUM") as p1p2:
                nhT_bf = p1s.tile([128, n_glob], BF, tag="big1")
                seT_bf = p1s.tile([128, n_glob], BF, tag="big2")
                qn = n_glob // 4
                for qq in range(4):
                    nc.sync.dma_start(out=nhT_bf[:, qn * qq:qn * (qq + 1)],
                                      in_=node_hT.ap()[:, qn * qq:qn * (qq + 1)])
                    nc.scalar.dma_start(out=seT_bf[:, qn * qq:qn * (qq + 1)],
                                        in_=seqT.ap()[:, qn * qq:qn * (qq + 1)])
                # table rows: node-major fp8, built 4 blocks (512 nodes) per
                # PSUM tile so the copy/write pipeline amortizes hop latency.
                tstage = p1s.tile([128, gblk, 256], F8, tag="tstage")
                for gq in range(gblk // 4):
                    ps4 = p1p2.tile([128, 4, 256], F32, tag="tps4")
                    for j in range(4):
                        b = 4 * gq + j
                        nc.tensor.matmul(out=ps4[:, j, 0:128],
                                         lhsT=nhT_bf[:, 128 * b:128 * (b + 1)],
                                         rhs=s_wb[:], start=True, stop=True)
                        nc.tensor.matmul(out=ps4[:, j, 128:256],
                                         lhsT=seT_bf[:, 128 * b:128 * (b + 1)],
                                         rhs=s_wd[:], start=True, stop=True)
                    if gq % 2 == 0:
                        nc.scalar.activation(out=tstage[:, 4 * gq:4 * gq + 4, :],
                                             in_=ps4[:], func=COPY)
                    else:
                        nc.vector.tensor_copy(out=tstage[:, 4 * gq:4 * gq + 4, :],
                                              in_=ps4[:])
                    if gq % 2 == 1:
                        g0 = 4 * (gq - 1)
                        tslice = table[128 * g0:128 * (g0 + 8), :]
                        tslice = tslice.rearrange("(b p) f -> p b f", p=128)
                        nc.sync.dma_start(out=tslice,
                                          in_=tstage[:, g0:g0 + 8, :])
                # aT = Wa^T h for local nodes (after the table: the table
                # gates the gathers, aT only gates chunk-0 mm1)
                for hh in range(n_loc // 512):
                    psa = p1p.tile([128, 512], F32, tag="psa")
                    nc.tensor.matmul(out=psa[:], lhsT=s_wa[:],
                                     rhs=nhT_bf[:, 512 * hh:512 * (hh + 1)],
                                     start=True, stop=True)
                    nc.scalar.activation(out=s_aT[:, 512 * hh:512 * (hh + 1)],
                                         in_=psa[:], func=COPY)

            # ---- phase 2: main edge loop, k-major chunks ----
            with tc.tile_pool(name="pp1", bufs=2, space="PSUM") as pp1, \
                 tc.tile_pool(name="ppw", bufs=2, space="PSUM") as ppw:
                def w2_stage(pend, hc):
                    t2s, t4p = pend
                    e0 = HC_E * hc
                    for b in range(3):
                        psw = ppw.tile([128, 512], F32, tag="psw",
                                       name=f"psw{hc}{b}")
                        nc.tensor.matmul(out=psw[:], lhsT=s_w2[:],
                                         rhs=t2s[hc][:, 512 * b:512 * (b + 1)],
                                         start=True, stop=True)
                        nc.scalar.activation(
                            out=t4p[:, e0 + 512 * b:e0 + 512 * (b + 1)],
                            in_=psw[:], func=GELU, bias=s_mb2c[:])

                def reduce_stage(pendx, _unused, chp):
                    # dense log-tree over the k-major layout (DVE; GpSimd's
                    # strict FIFO is owned by the gather instructions)
                    _, t4p = pendx[0]
                    r1 = p2r.tile([128, HC_E], BF, tag="r1")
                    nc.vector.tensor_add(out=r1[:], in0=t4p[:, 0:HC_E],
                                         in1=t4p[:, HC_E:CH_E])
                    nc.vector.tensor_add(out=r1[:, 0:768], in0=r1[:, 0:768],
                                         in1=r1[:, 768:1536])
                    nc.vector.tensor_add(out=r1[:, 0:384], in0=r1[:, 0:384],
                                         in1=r1[:, 384:768])
                    nc.vector.tensor_add(out=r1[:, 0:192], in0=r1[:, 0:192],
                                         in1=r1[:, 192:384])
                    nc.vector.tensor_add(out=r1[:, 0:64], in0=r1[:, 0:64],
                                         in1=r1[:, 64:128])
                    nc.vector.tensor_add(out=s_aggTb[:, CH_NODES * chp:
                                                     CH_NODES * (chp + 1)],
                                         in0=r1[:, 0:64], in1=r1[:, 128:192])

                pend = None
                for ch in range(n_ch):
                    # each chunk's gather split across swdge queues so
                    # descriptor generation runs in parallel; the first two
                    # chunks split 4 ways to cut the pipeline-fill latency
                    g = p2g.tile([128, N_SUB, 256], F8, tag="g")
                    nsplit = 4 if ch < 2 else 2
                    sub_s = N_SUB // nsplit
                    idx_s = (CH_E // 16) // nsplit
                    for hg in range(nsplit):
                        nc.gpsimd.dma_gather(
                            out_ap=g[:, sub_s * hg:sub_s * (hg + 1), :],
                            in_ap=table[:],
                            idxs_ap=s_idx[:, (CH_E // 16) * ch + idx_s * hg:
                                          (CH_E // 16) * ch + idx_s * (hg + 1)],
                            num_idxs=CH_E // nsplit,
                            num_idxs_reg=CH_E // nsplit,
                            elem_size=256,
                            single_packet=False,
                            queue_num=(2 * ch + hg) % 4,
                        )
                    e = p2e.tile([128, CH_E], F8, tag="e")
                    nc.sync.dma_start(out=e[:],
                                      in_=edge_hT.ap()[:, CH_E * ch:
                                                       CH_E * (ch + 1)])
                    # diag for this chunk (int16 select on DVE; GpSimd/Pool
                    # rejects int16 mult):
                    # dia_u16[p, s, n] = id01[p, n] * vpk[p, 24*ch + s];
                    # bitcast as fp8 pairs it is diag((1.0, m)) per subtile.
                    dia = p2d.tile([128, N_SUB, 128], I16, tag="dia")
                    idb = bass.AP(tensor=s_id01.tensor, offset=s_id01.offset,
                                  ap=[s_id01.ap[0], [0, N_SUB], s_id01.ap[1]])
                    vsl = s_vpk[:, N_SUB * ch:N_SUB * (ch + 1)]
                    vb = bass.AP(tensor=vsl.tensor, offset=vsl.offset,
                                 ap=[vsl.ap[0], vsl.ap[1], [0, 128]])
                    nc.vector.tensor_mul(out=dia[:], in0=idb, in1=vb)
                    dg8 = dia[:].bitcast(F8)

                    t4 = p2t4.tile([128, CH_E], BF, tag="t4")
                    na = s_aT[:, CH_NODES * ch:CH_NODES * (ch + 1)]
                    rep = bass.AP(tensor=na.tensor, offset=na.offset,
                                  ap=[na.ap[0], [0, 8], na.ap[1]])
                    t2s = []
                    for hc in range(2):
                        ps1 = pp1.tile([128, 3, 512], F32, tag="ps1")
                        e0 = HC_E * hc  # edge col offset within chunk
                        for b in range(3):
                            nc.tensor.matmul(
                                out=ps1[:, b, :], lhsT=s_wc[:],
                                rhs=e[:, e0 + 512 * b:e0 + 512 * (b + 1)],
                                start=True, stop=False)
                        for b in range(3):
                            nc.tensor.matmul(out=ps1[:, b, :], lhsT=s_id[:],
                                             rhs=rep, start=False, stop=False)
                        for sub in range(12):
                            gsub = 12 * hc + sub
                            bank = sub // 4
                            col = 128 * (sub % 4)
                            # fused DoubleRow: psum += h_j + m * seq_j in one
                            # matmul (planar lhsT pairs, interleaved rhs pairs)
                            gs = g[:, gsub, :]
                            lhsT = bass.AP(tensor=gs.tensor, offset=gs.offset,
                                           ap=[gs.ap[0], [128, 2], [1, 128]])
                            ds = dg8[:, gsub, :]
                            rhsd = bass.AP(tensor=ds.tensor, offset=ds.offset,
                                           ap=[ds.ap[0], [1, 2], [2, 128]])
                            nc.tensor.matmul(
                                out=ps1[:, bank, col:col + 128],
                                lhsT=lhsT, rhs=rhsd,
                                start=False, stop=True,
                                perf_mode=mybir.MatmulPerfMode.DoubleRow)
                        # previous chunk's w2 stage first: its gelu2 inputs
                        # are ready, so ACT drains them while PE works here.
                        if pend is not None:
                            w2_stage(pend[0], hc)
                            if hc == 1:
                                reduce_stage((pend[0],), None, pend[1])
                        t2 = p2t2.tile([128, HC_E], BF, tag="t2")
                        nc.scalar.activation(out=t2[:], in_=ps1[:], func=GELU,
                                             bias=s_mb1c[:])
                        t2s.append(t2)
                    pend = ((t2s, t4), ch)
                    # phase-3 head start for the first 512 nodes (chunks 0-7
                    # fully reduced by iteration 9): mm3, LN1, and the
                    # transpose back ride spare ppw psum slots mid-loop.
                    if ch == 10:
                        psm0 = ppw.tile([128, 512], F32, tag="psw",
                                        name="mm3h0")
                        nc.tensor.matmul(out=psm0[:], lhsT=s_w3[:],
                                         rhs=s_aggTb[:, 0:512],
                                         start=True, stop=True)
                        nc.scalar.activation(out=s_a2Tb[:, 0:512],
                                             in_=psm0[:], func=IDENT,
                                             bias=s_mb3x48[:])
                    if ch == 11:
                        psT0 = ppw.tile([128, 512], F32, tag="psw",
                                        name="lnT0")
                        v0 = psT0[:].rearrange("p (a b) -> p a b", a=4)
                        for b in range(4):
                            nc.tensor.matmul(
                                out=v0[:, b, :],
                                lhsT=s_a2Tb[:, 128 * b:128 * (b + 1)],
                                rhs=s_id[:], start=True, stop=True)
                        nc.vector.tensor_add(out=s_x1h[:], in0=v0,
                                             in1=s_nhl[:, 0:4, :])
                        sth = p2x.tile([128, 4, 6], F32, tag="sth")
                        for b in range(4):
                            nc.vector.bn_stats(out=sth[:, b, :],
                                               in_=s_x1h[:, b, :])
                            nc.vector.bn_aggr(out=s_mvh[:, b, :],
                                              in_=sth[:, b, :])
                w2_stage(pend[0], 0)
                w2_stage(pend[0], 1)
                reduce_stage((pend[0],), None, pend[1])

            for c in reversed(ctx2):
                c.__exit__(None, None, None)

            # ---- phase 3: mm3, LN1, FF, LN2, output (stage-batched) ----
            with tc.tile_pool(name="p3s", bufs=8) as p3s, \
                 tc.tile_pool(name="p3u", bufs=4) as p3u, \
                 tc.tile_pool(name="p3o", bufs=2) as p3o, \
                 tc.tile_pool(name="pp3", bufs=1, space="PSUM") as pp3, \
                 tc.tile_pool(name="ppm", bufs=1, space="PSUM") as ppm, \
                 tc.tile_pool(name="ppf", bufs=2, space="PSUM") as ppf, \
                 tc.tile_pool(name="ppf2", bufs=2, space="PSUM") as ppf2:
                nh_half = n_loc // 512
                # finish LN1 for the first half (sqrt deferred out of the
                # gelu-saturated loop so the sqrt table set loads once here)
                sdh = p3s.tile([128, 4], F32, tag="sdh")
                nc.scalar.activation(out=sdh[:], in_=s_mvh[:, :, 1:2],
                                     func=SQRT, bias=s_eps[:])
                rsh = p3s.tile([128, 4], F32, tag="rsh")
                nc.vector.reciprocal(out=rsh[:], in_=sdh[:])
                xnh = p3s.tile([128, 4, 128], BF, tag="xnh")
                for b in range(4):
                    nc.vector.tensor_scalar(out=xnh[:, b, :],
                                            in0=s_x1h[:, b, :],
                                            scalar1=s_mvh[:, b, 0:1],
                                            scalar2=rsh[:, b:b + 1],
                                            op0=SUB, op1=MUL)
                g1r4 = bass.AP(tensor=s_g1bc.tensor, offset=s_g1bc.offset,
                               ap=[s_g1bc.ap[0], [0, 4], s_g1bc.ap[1]])
                b1r4 = bass.AP(tensor=s_b1bc.tensor, offset=s_b1bc.offset,
                               ap=[s_b1bc.ap[0], [0, 4], s_b1bc.ap[1]])
                tbh = p3s.tile([128, 4, 128], BF, tag="tbh")
                nc.vector.tensor_mul(out=tbh[:], in0=xnh[:], in1=g1r4)
                nc.vector.tensor_add(out=s_h1rm[:, 0:4, :],
                                     in0=tbh[:], in1=b1r4)
                # mm3 + LN1 + transpose-back for the SECOND 512-node half
                psm = ppm.tile([128, 512], F32, tag="psm")
                nc.tensor.matmul(out=psm[:], lhsT=s_w3[:],
                                 rhs=s_aggTb[:, 512:1024],
                                 start=True, stop=True)
                nc.scalar.activation(out=s_a2Tb[:, 512:1024],
                                     in_=psm[:], func=IDENT,
                                     bias=s_mb3x48[:])
                pstT = pp3.tile([128, 4, 128], F32, tag="pstT")
                for b in range(4):
                    nc.tensor.matmul(
                        out=pstT[:, b, :],
                        lhsT=s_a2Tb[:, 512 + 128 * b:512 + 128 * (b + 1)],
                        rhs=s_id[:], start=True, stop=True)
                x1a = p3s.tile([128, 4, 128], F32, tag="x1a")
                nc.vector.tensor_add(out=x1a[:], in0=pstT[:],
                                     in1=s_nhl[:, 4:8, :])
                sta = p3s.tile([128, 4, 6], F32, tag="sta")
                mva = p3s.tile([128, 4, 2], F32, tag="mva")
                for b in range(4):
                    nc.vector.bn_stats(out=sta[:, b, :], in_=x1a[:, b, :])
                    nc.vector.bn_aggr(out=mva[:, b, :], in_=sta[:, b, :])
                sda = p3s.tile([128, 4], F32, tag="sda")
                nc.scalar.activation(out=sda[:], in_=mva[:, :, 1:2],
                                     func=SQRT, bias=s_eps[:])
                rstda = p3s.tile([128, 4], F32, tag="rstda")
                nc.vector.reciprocal(out=rstda[:], in_=sda[:])
                xna = p3s.tile([128, 4, 128], BF, tag="xna")
                for b in range(4):
                    nc.vector.tensor_scalar(out=xna[:, b, :],
                                            in0=x1a[:, b, :],
                                            scalar1=mva[:, b, 0:1],
                                            scalar2=rstda[:, b:b + 1],
                                            op0=SUB, op1=MUL)
                g1rep = bass.AP(tensor=s_g1bc.tensor, offset=s_g1bc.offset,
                                ap=[s_g1bc.ap[0], [0, 4], s_g1bc.ap[1]])
                b1rep = bass.AP(tensor=s_b1bc.tensor, offset=s_b1bc.offset,
                                ap=[s_b1bc.ap[0], [0, 4], s_b1bc.ap[1]])
                tb1a = p3s.tile([128, 4, 128], BF, tag="tb1a")
                nc.vector.tensor_mul(out=tb1a[:], in0=xna[:], in1=g1rep)
                nc.vector.tensor_add(out=s_h1rm[:, 4:8, :], in0=tb1a[:],
                                     in1=b1rep)
                psTT = pp3.tile([128, 4, 128], F32, tag="pstT", name="psTT2")
                for b in range(4):
                    nc.tensor.matmul(out=psTT[:, b, :],
                                     lhsT=s_h1rm[:, b, :],
                                     rhs=s_id[:], start=True, stop=True)
                nc.scalar.activation(out=s_h1T[:, 0:512],
                                     in_=psTT[:], func=COPY)
                psTT3 = pp3.tile([128, 4, 128], F32, tag="pstT", name="psTT3")
                for b in range(4):
                    nc.tensor.matmul(out=psTT3[:, b, :],
                                     lhsT=s_h1rm[:, 4 + b, :],
                                     rhs=s_id[:], start=True, stop=True)
                nc.scalar.activation(out=s_h1T[:, 512:1024],
                                     in_=psTT3[:], func=COPY)
                # FF + LN2 per 512-node half
                for hh in range(nh_half):
                    us = []
                    for fc in range(4):
                        psf = ppf.tile([128, 512], F32, tag="psf")
                        nc.tensor.matmul(out=psf[:],
                                         lhsT=s_fw1[:, 128 * fc:128 * (fc + 1)],
                                         rhs=s_h1T[:, 512 * hh:512 * (hh + 1)],
                                         start=True, stop=True)
                        u = p3u.tile([128, 512], BF, tag=f"u{fc}")
                        nc.scalar.activation(out=u[:], in_=psf[:], func=GELU,
                                             bias=s_fb1c[:, fc:fc + 1])
                        us.append(u)
                    psf2 = ppf2.tile([128, 512], F32, tag="psf2")
                    for fc in range(4):
                        nc.tensor.matmul(out=psf2[:], lhsT=s_fw2[:, fc, :],
                                         rhs=us[fc][:], start=(fc == 0),
                                         stop=(fc == 3))
                    ffT4 = p3s.tile([128, 4, 128], BF, tag="ffT4")
                    nc.scalar.activation(out=ffT4[:], in_=psf2[:],
                                         func=IDENT, bias=s_fb2c[:])
                    psr2T = pp3.tile([128, 4, 128], F32, tag="psr2T")
                    for j in range(4):
                        nc.tensor.matmul(out=psr2T[:, j, :],
                                         lhsT=ffT4[:, j, :],
                                         rhs=s_id[:], start=True, stop=True)
                    x2a = p3s.tile([128, 4, 128], F32, tag="x2a")
                    nc.vector.tensor_add(out=x2a[:], in0=psr2T[:],
                                         in1=s_h1rm[:, 4 * hh:4 * hh + 4, :])
                    st2a = p3s.tile([128, 4, 6], F32, tag="st2a")
                    mv2a = p3s.tile([128, 4, 2], F32, tag="mv2a")
                    for j in range(4):
                        nc.vector.bn_stats(out=st2a[:, j, :], in_=x2a[:, j, :])
                        nc.vector.bn_aggr(out=mv2a[:, j, :], in_=st2a[:, j, :])
                    sd2a = p3s.tile([128, 4], F32, tag="sd2a")
                    nc.scalar.activation(out=sd2a[:], in_=mv2a[:, :, 1:2],
                                         func=SQRT, bias=s_eps[:])
                    rstd2a = p3s.tile([128, 4], F32, tag="rstd2a")
                    nc.vector.reciprocal(out=rstd2a[:], in_=sd2a[:])
                    xn2a = p3s.tile([128, 4, 128], F32, tag="xn2a")
                    for j in range(4):
                        nc.vector.tensor_scalar(out=xn2a[:, j, :],
                                                in0=x2a[:, j, :],
                                                scalar1=mv2a[:, j, 0:1],
                                                scalar2=rstd2a[:, j:j + 1],
                                                op0=SUB, op1=MUL)
                    g2rep = bass.AP(tensor=s_g2bc.tensor, offset=s_g2bc.offset,
                                    ap=[s_g2bc.ap[0], [0, 4], s_g2bc.ap[1]])
                    b2rep = bass.AP(tensor=s_b2bc.tensor, offset=s_b2bc.offset,
                                    ap=[s_b2bc.ap[0], [0, 4], s_b2bc.ap[1]])
                    ob = p3o.tile([128, 4, 128], F32, tag="ob")
                    tga = p3s.tile([128, 4, 128], F32, tag="tga")
                    nc.vector.tensor_mul(out=tga[:], in0=xn2a[:], in1=g2rep)
                    nc.vector.tensor_add(out=ob[:], in0=tga[:], in1=b2rep)
                    oslice = out.ap()[512 * hh:512 * (hh + 1), :]
                    oslice = oslice.rearrange("(j p) f -> p j f", p=128)
                    nc.sync.dma_start(out=oslice, in_=ob[:])

    nc.compile()
    return nc


def prep_core_inputs(inputs, n_glob, n_loc, core):
    """Host-side layout prep for one core: slicing, k-major reorder within
    chunks, transposes, dtype casts, tiny constant broadcasts. No kernel math
    (no indexing of data tensors by edge_idx) is done on the host."""
    f32 = np.float32
    n0 = core * n_loc
    E = n_loc * KK
    n_ch = E // CH_E

    def kmajor(x):
        # x: [n_loc, KK, ...] -> per 64-node chunk: [KK, 64, ...] -> flat E
        tail = x.shape[2:]
        x = x.reshape(n_ch, CH_NODES, KK, *tail)
        x = x.transpose(0, 2, 1, *range(3, 3 + len(tail)))
        return np.ascontiguousarray(x.reshape(E, *tail))

    eh = kmajor(inputs["edge_h"][n0:n0 + n_loc])          # [E, H] k-major
    eh = np.ascontiguousarray(eh.T).astype(FP8)           # [H, E] fp8
    j = (inputs["edge_idx"][n0:n0 + n_loc].astype(np.int64) - n0) % n_glob
    j = kmajor(j)                                         # [E] k-major
    idx16 = np.tile(np.ascontiguousarray(j.reshape(E // 16, 16).T), (8, 1)
                    ).astype(np.int16)
    m = kmajor(inputs["ar_mask"][n0:n0 + n_loc])          # [E] k-major
    m8 = np.ascontiguousarray(m.reshape(E // 128, 128).T).astype(FP8)
    one8 = np.float32(1.0).astype(FP8)
    vpk = ((m8.view(np.uint8).astype(np.uint16) << 8)
           | np.uint16(one8.view(np.uint8))).view(np.int16)
    node_hT = np.ascontiguousarray(
        np.roll(inputs["node_h"], -n0, axis=0).T).astype(BF16)
    seqT = np.ascontiguousarray(
        np.roll(inputs["seq_emb"], -n0, axis=0).T).astype(BF16)
    nhl = np.ascontiguousarray(
        inputs["node_h"][n0:n0 + n_loc].reshape(n_loc // 128, 128, H)
        .transpose(1, 0, 2)).astype(f32)
    mW1 = inputs["mW1"]
    bb = np.zeros((128, 2176), dtype=BF16)
    bb[:, 0:128] = mW1[0:128].astype(BF16)
    bb[:, 128:256] = mW1[128:256].astype(BF16)
    bb[:, 256:384] = mW1[384:512].astype(BF16)
    bb[:, 384:512] = mW1[256:384].astype(BF16)
    bb[:, 512:640] = inputs["mW2"].astype(BF16)
    bb[:, 640:768] = inputs["mW3"].astype(BF16)
    bb[:, 768:1280] = inputs["fW1"].astype(BF16)
    bb[:, 1280:1792] = np.ascontiguousarray(
        inputs["fW2"].reshape(4, 128, H).transpose(1, 0, 2)).reshape(
        128, 512).astype(BF16)
    bb[:, 1792:1920] = np.eye(128, dtype=BF16)
    bb[:, 1920:2048] = np.tile(inputs["g1"][None, :], (128, 1)).astype(BF16)
    bb[:, 2048:2176] = np.tile(inputs["b1"][None, :], (128, 1)).astype(BF16)
    bf = np.zeros((128, 1288), dtype=f32)
    bf[:, 0] = inputs["mb1"].astype(f32)
    bf[:, 1] = inputs["mb2"].astype(f32)
    bf[:, 2] = (inputs["mb3"] * KK).astype(f32)
    bf[:, 3:7] = np.ascontiguousarray(inputs["fb1"].reshape(4, 128).T)
    bf[:, 7] = inputs["fb2"].astype(f32)
    bf[:, 8:136] = np.tile(inputs["g2"][None, :], (128, 1))
    bf[:, 136:264] = np.tile(inputs["b2"][None, :], (128, 1))
    bf[:, 264:1288] = nhl.reshape(128, 1024)
    d = {
        "edge_hT": eh, "idx16": idx16,
        "node_hT": node_hT, "seqT": seqT,
        "blob_bf": bb, "blob_f32": bf,
        "id01": np.eye(128, dtype=np.int16), "vpk": vpk,
    }
    return d


_NC_CACHE = {}


def kernel(**inputs):
    inputs = {k: np.asarray(v) for k, v in inputs.items()}
    n_glob = inputs["node_h"].shape[0]
    n_loc = n_glob // N_CORES
    key = (n_glob, n_loc)
    if key not in _NC_CACHE:
        _NC_CACHE[key] = build_nc(n_glob, n_loc, N_CORES)
    nc = _NC_CACHE[key]
    in_maps = [prep_core_inputs(inputs, n_glob, n_loc, c)
               for c in range(N_CORES)]
    res = bass_utils.run_bass_kernel_spmd(nc, in_maps,
                                          core_ids=list(range(N_CORES)))
    return np.concatenate([res.results[c]["out"] for c in range(N_CORES)],
                          axis=0).astype(np.float32)


# revision 38
# speedup vs baseline: 1.2330x; 1.0033x over previous
"""Trainium2 Bass kernel for nn_DecoderLayer (GNN message passing decoder layer).

Math (per reference):
  seq_j = seq_emb[edge_idx] * ar_mask[..., None]
  x = concat([h_i, h_j, edge_h, seq_j], -1)            # [res,k,4h]
  msg = gelu(x @ mW1 + mb1); msg = gelu(msg @ mW2 + mb2); msg = msg @ mW3 + mb3
  agg = msg.sum(1)
  h = LN(node_h + agg) * g1 + b1
  ff = gelu(h @ fW1 + fb1) @ fW2 + fb2
  h = LN(h + ff) * g2 + b2

Strategy (8-way data parallel over the residue dim, no collectives):
  - mm1 decomposed: x@mW1 = h_i@Wa + h_j@Wb + edge_h@Wc + seq_j@Wd.
    h_j@Wb and seq_emb@Wd are precomputed per global node into a fused FP8
    gather table [8192, 256] in DRAM; per-edge rows fetched with dma_gather
    (256B packets - half the DMA-engine packet cost of bf16).
  - edge_h streamed in fp8 as well (mixed fp8xbf16 matmuls keep the weights
    in bf16, so only per-edge data pays fp8 quantization; predicted rel err
    ~5e-3 vs the 2e-2 gate).
  - dma_gather descriptor generation runs on the GpSimd Q7 core-pair selected
    by queue_num; queues 1-3 run concurrently on three disjoint pairs.
  - Edges are k-major within 3072-edge chunks (64 nodes x 48 k): col=k*64+n.
    Aligns 512-col matmul slices with the per-node h_i@Wa broadcast AP and
    makes the k-reduction a log-tree of dense bf16 adds on DVE.
  - ar_mask folded into the PE transpose of the gathered seq half via
    rhs = diag(mask); the diag tiles are built ON DEVICE per chunk with one
    stride-0 int16 DVE multiply (select) from a 96KB packed mask input --
    replaces the 12.6MB host-built diag tensor of the earlier version.
  - k-reduction before mm3 (linearity): 48x less mm3 work.
  - mm2 (+gelu2+reduce) of chunk N is issued interleaved into chunk N+1's
    mm1 stream so ACT always has ready work and PE never waits on gelu1.
"""

import os
import sys

sys.path.insert(0, "/opt/trn_rl_repo")

import numpy as np
import ml_dtypes

import concourse.bacc as bacc
import concourse.bass as bass
import concourse.mybir as mybir
import concourse.tile as tile
from concourse import bass_utils

BF16 = ml_dtypes.bfloat16
FP8 = ml_dtypes.float8_e4m3
F32 = mybir.dt.float32
BF = mybir.dt.bfloat16
I16 = mybir.dt.int16
F8 = mybir.dt.float8e4

RES, KK, H = 8192, 48, 128
N_CORES = 8
CH_NODES = 64                 # nodes per chunk
CH_E = CH_NODES * KK          # 3072 edges per chunk
HC_E = CH_E // 2              # 1536 edges per half-chunk
N_SUB = CH_E // 128           # 24 subtiles of 128 edges per chunk


def build_nc(n_glob, n_loc, num_devices):
    E = n_loc * KK
    n_ch = E // CH_E           # 16 chunks
    nblk = n_loc // 128        # 8 local node blocks
    gblk = n_glob // 128       # 64 global node blocks

    nc = bacc.Bacc("TRN2", target_bir_lowering=False, debug=False,
                   num_devices=num_devices, num_swdge_queues=4)

    def din(name, shape, dt):
        return nc.dram_tensor(name, shape, dt, kind="ExternalInput")

    edge_hT = din("edge_hT", [H, E], F8)            # k-major per chunk, fp8
    idx16 = din("idx16", [128, E // 16], I16)       # k-major per chunk
    node_hT = din("node_hT", [H, n_glob], BF)       # rotated: local first
    seqT = din("seqT", [H, n_glob], BF)
    blob_bf = din("blob_bf", [128, 2176], BF)       # packed bf16 constants
    blob_f32 = din("blob_f32", [128, 1288], F32)    # packed f32 constants
    id01 = din("id01", [128, 128], I16)             # identity as int 0/1
    # vpk[p, s] = (fp8bits(mask) << 8) | fp8bits(1.0): one u16 per edge whose
    # int16-select against id01 yields the interleaved (1.0, m) fp8 pair diag
    # consumed as the DoubleRow rhs.
    vpk = din("vpk", [128, E // 128], I16)
    out = nc.dram_tensor("out", [n_loc, H], F32, kind="ExternalOutput")

    GELU = mybir.ActivationFunctionType.Gelu
    IDENT = mybir.ActivationFunctionType.Identity
    COPY = mybir.ActivationFunctionType.Copy
    SQRT = mybir.ActivationFunctionType.Sqrt
    SUB = mybir.AluOpType.subtract
    MUL = mybir.AluOpType.mult

    with tile.TileContext(nc) as tc:
        with tc.tile_pool(name="singles", bufs=1) as sg, \
             tc.tile_pool(name="dram", bufs=1, space="DRAM") as dp:
            # ---- resident constants: two packed blobs + idx + mask ----
            s_bb = sg.tile([128, 2176], BF)
            nc.sync.dma_start(out=s_bb[:], in_=blob_bf.ap())
            s_bf = sg.tile([128, 1288], F32)
            nc.scalar.dma_start(out=s_bf[:], in_=blob_f32.ap())
            s_idx = sg.tile([128, E // 16], I16)
            nc.sync.dma_start(out=s_idx[:], in_=idx16.ap())
            s_id01 = sg.tile([128, 128], I16)
            nc.scalar.dma_start(out=s_id01[:], in_=id01.ap())
            s_vpk = sg.tile([128, E // 128], I16)
            nc.scalar.dma_start(out=s_vpk[:], in_=vpk.ap())
            s_wa = s_bb[:, 0:128]
            s_wb = s_bb[:, 128:256]
            s_wd = s_bb[:, 256:384]
            s_wc = s_bb[:, 384:512]
            s_w2 = s_bb[:, 512:640]
            s_w3 = s_bb[:, 640:768]
            s_fw1 = s_bb[:, 768:1280]
            s_fw2 = s_bb[:, 1280:1792].rearrange("p (a b) -> p a b", a=4)
            s_id = s_bb[:, 1792:1920]
            s_g1bc = s_bb[:, 1920:2048]
            s_b1bc = s_bb[:, 2048:2176]
            s_mb1c = s_bf[:, 0:1]
            s_mb2c = s_bf[:, 1:2]
            s_mb3x48 = s_bf[:, 2:3]
            s_fb1c = s_bf[:, 3:7]
            s_fb2c = s_bf[:, 7:8]
            s_g2bc = s_bf[:, 8:136]
            s_b2bc = s_bf[:, 136:264]
            s_nhl = s_bf[:, 264:1288].rearrange("p (a b) -> p a b", a=nblk)
            s_eps = sg.tile([128, 1], F32)
            nc.vector.memset(s_eps[:], 1e-5)

            s_x1h = sg.tile([128, 4, 128], F32)     # hoisted LN1-h0 input
            s_mvh = sg.tile([128, 4, 2], F32)       # hoisted LN1-h0 mean/var
            s_aT = sg.tile([128, n_loc], BF)        # (Wa^T h_i) per local node
            s_aggTb = sg.tile([128, n_loc], BF)     # k-sum of msg2, fm bf16
            s_a2Tb = sg.tile([128, n_loc], BF)
            s_h1T = sg.tile([128, n_loc], BF)
            s_h1rm = sg.tile([128, nblk, H], BF)

            table = dp.tile([n_glob, 256], F8)

            # ---- phase 1: gather table (fp8) + Wa precompute ----
            # (phase-2 SBUF pools open first so their zone sits below the
            # phase-1 pools on the allocator stack: phase-2 DMAs can then
            # start during phase 1 instead of waiting for its release)
            ctx2 = [tc.tile_pool(name="p2g", bufs=6),
                    tc.tile_pool(name="p2e", bufs=3),
                    tc.tile_pool(name="p2d", bufs=3),
                    tc.tile_pool(name="p2t2", bufs=4),
                    tc.tile_pool(name="p2r", bufs=2),
                    tc.tile_pool(name="p2t4", bufs=3),
                    tc.tile_pool(name="p2x", bufs=1)]
            p2g, p2e, p2d, p2t2, p2r, p2t4, p2x = [c.__enter__() for c in ctx2]
            with tc.tile_pool(name="p1s", bufs=1) as p1s, \
                 tc.tile_pool(name="p1p", bufs=2, space="PSUM") as p1p, \
                 tc.tile_pool(name="p1p2", bufs=3, space="PSUM") as p1p2:
                nhT_bf = p1s.tile([128, n_glob], BF, tag="big1")
                seT_bf = p1s.tile([128, n_glob], BF, tag="big2")
                qn = n_glob // 4
                for qq in range(4):
                    nc.sync.dma_start(out=nhT_bf[:, qn * qq:qn * (qq + 1)],
                                      in_=node_hT.ap()[:, qn * qq:qn * (qq + 1)])
                    nc.scalar.dma_start(out=seT_bf[:, qn * qq:qn * (qq + 1)],
                                        in_=seqT.ap()[:, qn * qq:qn * (qq + 1)])
                # table rows: node-major fp8, built 4 blocks (512 nodes) per
                # PSUM tile so the copy/write pipeline amortizes hop latency.
                tstage = p1s.tile([128, gblk, 256], F8, tag="tstage")
                for gq in range(gblk // 4):
                    ps4 = p1p2.tile([128, 4, 256], F32, tag="tps4")
                    for j in range(4):
                        b = 4 * gq + j
                        nc.tensor.matmul(out=ps4[:, j, 0:128],
                                         lhsT=nhT_bf[:, 128 * b:128 * (b + 1)],
                                         rhs=s_wb[:], start=True, stop=True)
                        nc.tensor.matmul(out=ps4[:, j, 128:256],
                                         lhsT=seT_bf[:, 128 * b:128 * (b + 1)],
                                         rhs=s_wd[:], start=True, stop=True)
                    if gq % 2 == 0:
                        nc.scalar.activation(out=tstage[:, 4 * gq:4 * gq + 4, :],
                                             in_=ps4[:], func=COPY)
                    else:
                        nc.vector.tensor_copy(out=tstage[:, 4 * gq:4 * gq + 4, :],
                                              in_=ps4[:])
                    if gq % 2 == 1:
                        g0 = 4 * (gq - 1)
                        tslice = table[128 * g0:128 * (g0 + 8), :]
                        tslice = tslice.rearrange("(b p) f -> p b f", p=128)
                        nc.sync.dma_start(out=tslice,
                                          in_=tstage[:, g0:g0 + 8, :])
                # aT = Wa^T h for local nodes (after the table: the table
                # gates the gathers, aT only gates chunk-0 mm1)
                for hh in range(n_loc // 512):
                    psa = p1p.tile([128, 512], F32, tag="psa")
                    nc.tensor.matmul(out=psa[:], lhsT=s_wa[:],
                                     rhs=nhT_bf[:, 512 * hh:512 * (hh + 1)],
                                     start=True, stop=True)
                    nc.scalar.activation(out=s_aT[:, 512 * hh:512 * (hh + 1)],
                                         in_=psa[:], func=COPY)

            # ---- phase 2: main edge loop, k-major chunks ----
            with tc.tile_pool(name="pp1", bufs=2, space="PSUM") as pp1, \
                 tc.tile_pool(name="ppw", bufs=2, space="PSUM") as ppw:
                def w2_stage(pend, hc):
                    t2s, t4p = pend
                    e0 = HC_E * hc
                    for b in range(3):
                        psw = ppw.tile([128, 512], F32, tag="psw",
                                       name=f"psw{hc}{b}")
                        nc.tensor.matmul(out=psw[:], lhsT=s_w2[:],
                                         rhs=t2s[hc][:, 512 * b:512 * (b + 1)],
                                         start=True, stop=True)
                        nc.scalar.activation(
                            out=t4p[:, e0 + 512 * b:e0 + 512 * (b + 1)],
                            in_=psw[:], func=GELU, bias=s_mb2c[:])

                def reduce_stage(pendx, _unused, chp):
                    # dense log-tree over the k-major layout (DVE; GpSimd's
                    # strict FIFO is owned by the gather instructions)
                    _, t4p = pendx[0]
                    r1 = p2r.tile([128, HC_E], BF, tag="r1")
                    nc.vector.tensor_add(out=r1[:], in0=t4p[:, 0:HC_E],
                                         in1=t4p[:, HC_E:CH_E])
                    nc.vector.tensor_add(out=r1[:, 0:768], in0=r1[:, 0:768],
                                         in1=r1[:, 768:1536])
                    nc.vector.tensor_add(out=r1[:, 0:384], in0=r1[:, 0:384],
                                         in1=r1[:, 384:768])
                    nc.vector.tensor_add(out=r1[:, 0:192], in0=r1[:, 0:192],
                                         in1=r1[:, 192:384])
                    nc.vector.tensor_add(out=r1[:, 0:64], in0=r1[:, 0:64],
                                         in1=r1[:, 64:128])
                    nc.vector.tensor_add(out=s_aggTb[:, CH_NODES * chp:
                                                     CH_NODES * (chp + 1)],
                                         in0=r1[:, 0:64], in1=r1[:, 128:192])

                pend = None
                for ch in range(n_ch):
                    # each chunk's gather split across swdge queues so
                    # descriptor generation runs in parallel; the first two
                    # chunks split 4 ways to cut the pipeline-fill latency
                    g = p2g.tile([128, N_SUB, 256], F8, tag="g")
                    nsplit = 4 if ch < 2 else 2
                    sub_s = N_SUB // nsplit
                    idx_s = (CH_E // 16) // nsplit
                    for hg in range(nsplit):
                        nc.gpsimd.dma_gather(
                            out_ap=g[:, sub_s * hg:sub_s * (hg + 1), :],
                            in_ap=table[:],
                            idxs_ap=s_idx[:, (CH_E // 16) * ch + idx_s * hg:
                                          (CH_E // 16) * ch + idx_s * (hg + 1)],
                            num_idxs=CH_E // nsplit,
                            num_idxs_reg=CH_E // nsplit,
                            elem_size=256,
                            single_packet=False,
                            queue_num=(2 * ch + hg) % 4,
                        )
                    e = p2e.tile([128, CH_E], F8, tag="e")
                    nc.sync.dma_start(out=e[:],
                                      in_=edge_hT.ap()[:, CH_E * ch:
                                                       CH_E * (ch + 1)])
                    # diag for this chunk (int16 select on DVE; GpSimd/Pool
                    # rejects int16 mult):
                    # dia_u16[p, s, n] = id01[p, n] * vpk[p, 24*ch + s];
                    # bitcast as fp8 pairs it is diag((1.0, m)) per subtile.
                    dia = p2d.tile([128, N_SUB, 128], I16, tag="dia")
                    idb = bass.AP(tensor=s_id01.tensor, offset=s_id01.offset,
                                  ap=[s_id01.ap[0], [0, N_SUB], s_id01.ap[1]])
                    vsl = s_vpk[:, N_SUB * ch:N_SUB * (ch + 1)]
                    vb = bass.AP(tensor=vsl.tensor, offset=vsl.offset,
                                 ap=[vsl.ap[0], vsl.ap[1], [0, 128]])
                    nc.vector.tensor_mul(out=dia[:], in0=idb, in1=vb)
                    dg8 = dia[:].bitcast(F8)

                    t4 = p2t4.tile([128, CH_E], BF, tag="t4")
                    na = s_aT[:, CH_NODES * ch:CH_NODES * (ch + 1)]
                    rep = bass.AP(tensor=na.tensor, offset=na.offset,
                                  ap=[na.ap[0], [0, 8], na.ap[1]])
                    t2s = []
                    for hc in range(2):
                        ps1 = pp1.tile([128, 3, 512], F32, tag="ps1")
                        e0 = HC_E * hc  # edge col offset within chunk
                        for b in range(3):
                            nc.tensor.matmul(
                                out=ps1[:, b, :], lhsT=s_wc[:],
                                rhs=e[:, e0 + 512 * b:e0 + 512 * (b + 1)],
                                start=True, stop=False)
                        for b in range(3):
                            nc.tensor.matmul(out=ps1[:, b, :], lhsT=s_id[:],
                                             rhs=rep, start=False, stop=False)
                        for sub in range(12):
                            gsub = 12 * hc + sub
                            bank = sub // 4
                            col = 128 * (sub % 4)
                            # fused DoubleRow: psum += h_j + m * seq_j in one
                            # matmul (planar lhsT pairs, interleaved rhs pairs)
                            gs = g[:, gsub, :]
                            lhsT = bass.AP(tensor=gs.tensor, offset=gs.offset,
                                           ap=[gs.ap[0], [128, 2], [1, 128]])
                            ds = dg8[:, gsub, :]
                            rhsd = bass.AP(tensor=ds.tensor, offset=ds.offset,
                                           ap=[ds.ap[0], [1, 2], [2, 128]])
                            nc.tensor.matmul(
                                out=ps1[:, bank, col:col + 128],
                                lhsT=lhsT, rhs=rhsd,
                                start=False, stop=True,
                                perf_mode=mybir.MatmulPerfMode.DoubleRow)
                        # previous chunk's w2 stage first: its gelu2 inputs
                        # are ready, so ACT drains them while PE works here.
                        if pend is not None:
                            w2_stage(pend[0], hc)
                            if hc == 1:
                                reduce_stage((pend[0],), None, pend[1])
                        t2 = p2t2.tile([128, HC_E], BF, tag="t2")
                        nc.scalar.activation(out=t2[:], in_=ps1[:], func=GELU,
                                             bias=s_mb1c[:])
                        t2s.append(t2)
                    pend = ((t2s, t4), ch)
                    # phase-3 head start for the first 512 nodes (chunks 0-7
                    # fully reduced by iteration 9): mm3, LN1, and the
                    # transpose back ride spare ppw psum slots mid-loop.
                    if ch == 10:
                        psm0 = ppw.tile([128, 512], F32, tag="psw",
                                        name="mm3h0")
                        nc.tensor.matmul(out=psm0[:], lhsT=s_w3[:],
                                         rhs=s_aggTb[:, 0:512],
                                         start=True, stop=True)
                        nc.scalar.activation(out=s_a2Tb[:, 0:512],
                                             in_=psm0[:], func=IDENT,
                                             bias=s_mb3x48[:])
                    if ch == 11:
                        psT0 = ppw.tile([128, 512], F32, tag="psw",
                                        name="lnT0")
                        v0 = psT0[:].rearrange("p (a b) -> p a b", a=4)
                        for b in range(4):
                            nc.tensor.matmul(
                                out=v0[:, b, :],
                                lhsT=s_a2Tb[:, 128 * b:128 * (b + 1)],
                                rhs=s_id[:], start=True, stop=True)
                        nc.vector.tensor_add(out=s_x1h[:], in0=v0,
                                             in1=s_nhl[:, 0:4, :])
                        sth = p2x.tile([128, 4, 6], F32, tag="sth")
                        for b in range(4):
                            nc.vector.bn_stats(out=sth[:, b, :],
                                               in_=s_x1h[:, b, :])
                            nc.vector.bn_aggr(out=s_mvh[:, b, :],
                                              in_=sth[:, b, :])
                w2_stage(pend[0], 0)
                w2_stage(pend[0], 1)
                reduce_stage((pend[0],), None, pend[1])

            for c in reversed(ctx2):
                c.__exit__(None, None, None)

            # ---- phase 3: mm3, LN1, FF, LN2, output (stage-batched) ----
            with tc.tile_pool(name="p3s", bufs=8) as p3s, \
                 tc.tile_pool(name="p3u", bufs=4) as p3u, \
                 tc.tile_pool(name="p3o", bufs=2) as p3o, \
                 tc.tile_pool(name="pp3", bufs=1, space="PSUM") as pp3, \
                 tc.tile_pool(name="ppm", bufs=1, space="PSUM") as ppm, \
                 tc.tile_pool(name="ppf", bufs=2, space="PSUM") as ppf, \
                 tc.tile_pool(name="ppf2", bufs=2, space="PSUM") as ppf2:
                nh_half = n_loc // 512
                # finish LN1 for the first half (sqrt deferred out of the
                # gelu-saturated loop so the sqrt table set loads once here)
                sdh = p3s.tile([128, 4], F32, tag="sdh")
                nc.scalar.activation(out=sdh[:], in_=s_mvh[:, :, 1:2],
                                     func=SQRT, bias=s_eps[:])
                rsh = p3s.tile([128, 4], F32, tag="rsh")
                nc.vector.reciprocal(out=rsh[:], in_=sdh[:])
                xnh = p3s.tile([128, 4, 128], BF, tag="xnh")
                for b in range(4):
                    nc.vector.tensor_scalar(out=xnh[:, b, :],
                                            in0=s_x1h[:, b, :],
                                            scalar1=s_mvh[:, b, 0:1],
                                            scalar2=rsh[:, b:b + 1],
                                            op0=SUB, op1=MUL)
                g1r4 = bass.AP(tensor=s_g1bc.tensor, offset=s_g1bc.offset,
                               ap=[s_g1bc.ap[0], [0, 4], s_g1bc.ap[1]])
                b1r4 = bass.AP(tensor=s_b1bc.tensor, offset=s_b1bc.offset,
                               ap=[s_b1bc.ap[0], [0, 4], s_b1bc.ap[1]])
                tbh = p3s.tile([128, 4, 128], BF, tag="tbh")
                nc.vector.tensor_mul(out=tbh[:], in0=xnh[:], in1=g1r4)
                nc.vector.tensor_add(out=s_h1rm[:, 0:4, :],
                                     in0=tbh[:], in1=b1r4)
                # mm3 + LN1 + transpose-back for the SECOND 512-node half
                psm = ppm.tile([128, 512], F32, tag="psm")
                nc.tensor.matmul(out=psm[:], lhsT=s_w3[:],
                                 rhs=s_aggTb[:, 512:1024],
                                 start=True, stop=True)
                nc.scalar.activation(out=s_a2Tb[:, 512:1024],
                                     in_=psm[:], func=IDENT,
                                     bias=s_mb3x48[:])
                pstT = pp3.tile([128, 4, 128], F32, tag="pstT")
                for b in range(4):
                    nc.tensor.matmul(
                        out=pstT[:, b, :],
                        lhsT=s_a2Tb[:, 512 + 128 * b:512 + 128 * (b + 1)],
                        rhs=s_id[:], start=True, stop=True)
                x1a = p3s.tile([128, 4, 128], F32, tag="x1a")
                nc.vector.tensor_add(out=x1a[:], in0=pstT[:],
                                     in1=s_nhl[:, 4:8, :])
                sta = p3s.tile([128, 4, 6], F32, tag="sta")
                mva = p3s.tile([128, 4, 2], F32, tag="mva")
                for b in range(4):
                    nc.vector.bn_stats(out=sta[:, b, :], in_=x1a[:, b, :])
                    nc.vector.bn_aggr(out=mva[:, b, :], in_=sta[:, b, :])
                sda = p3s.tile([128, 4], F32, tag="sda")
                nc.scalar.activation(out=sda[:], in_=mva[:, :, 1:2],
                                     func=SQRT, bias=s_eps[:])
                rstda = p3s.tile([128, 4], F32, tag="rstda")
                nc.vector.reciprocal(out=rstda[:], in_=sda[:])
                xna = p3s.tile([128, 4, 128], BF, tag="xna")
                for b in range(4):
                    nc.vector.tensor_scalar(out=xna[:, b, :],
                                            in0=x1a[:, b, :],
                                            scalar1=mva[:, b, 0:1],
                                            scalar2=rstda[:, b:b + 1],
                                            op0=SUB, op1=MUL)
                g1rep = bass.AP(tensor=s_g1bc.tensor, offset=s_g1bc.offset,
                                ap=[s_g1bc.ap[0], [0, 4], s_g1bc.ap[1]])
                b1rep = bass.AP(tensor=s_b1bc.tensor, offset=s_b1bc.offset,
                                ap=[s_b1bc.ap[0], [0, 4], s_b1bc.ap[1]])
                tb1a = p3s.tile([128, 4, 128], BF, tag="tb1a")
                nc.vector.tensor_mul(out=tb1a[:], in0=xna[:], in1=g1rep)
                nc.vector.tensor_add(out=s_h1rm[:, 4:8, :], in0=tb1a[:],
                                     in1=b1rep)
                psTT = pp3.tile([128, 4, 128], F32, tag="pstT", name="psTT2")
                for b in range(4):
                    nc.tensor.matmul(out=psTT[:, b, :],
                                     lhsT=s_h1rm[:, b, :],
                                     rhs=s_id[:], start=True, stop=True)
                nc.scalar.activation(out=s_h1T[:, 0:512],
                                     in_=psTT[:], func=COPY)
                psTT3 = pp3.tile([128, 4, 128], F32, tag="pstT", name="psTT3")
                for b in range(4):
                    nc.tensor.matmul(out=psTT3[:, b, :],
                                     lhsT=s_h1rm[:, 4 + b, :],
                                     rhs=s_id[:], start=True, stop=True)
                nc.scalar.activation(out=s_h1T[:, 512:1024],
                                     in_=psTT3[:], func=COPY)
                # FF for both halves first (one gelu table-set window),
                # then both LN2 sqrts adjacent (one sqrt window): avoids two
                # more ~2.7us ACT table reloads in the tail.
                ffparts = []
                for hh in range(nh_half):
                    us = []
                    for fc in range(4):
                        psf = ppf.tile([128, 512], F32, tag="psf")
                        nc.tensor.matmul(out=psf[:],
                                         lhsT=s_fw1[:, 128 * fc:128 * (fc + 1)],
                                         rhs=s_h1T[:, 512 * hh:512 * (hh + 1)],
                                         start=True, stop=True)
                        u = p3u.tile([128, 512], BF, tag=f"u{fc}")
                        nc.scalar.activation(out=u[:], in_=psf[:], func=GELU,
                                             bias=s_fb1c[:, fc:fc + 1])
                        us.append(u)
                    psf2 = ppf2.tile([128, 512], F32, tag="psf2")
                    for fc in range(4):
                        nc.tensor.matmul(out=psf2[:], lhsT=s_fw2[:, fc, :],
                                         rhs=us[fc][:], start=(fc == 0),
                                         stop=(fc == 3))
                    ffT4 = p3s.tile([128, 4, 128], BF, tag="ffT4")
                    nc.scalar.activation(out=ffT4[:], in_=psf2[:],
                                         func=IDENT, bias=s_fb2c[:])
                    psr2T = pp3.tile([128, 4, 128], F32, tag="psr2T")
                    for j in range(4):
                        nc.tensor.matmul(out=psr2T[:, j, :],
                                         lhsT=ffT4[:, j, :],
                                         rhs=s_id[:], start=True, stop=True)
                    x2a = p3s.tile([128, 4, 128], F32, tag="x2a")
                    nc.vector.tensor_add(out=x2a[:], in0=psr2T[:],
                                         in1=s_h1rm[:, 4 * hh:4 * hh + 4, :])
                    st2a = p3s.tile([128, 4, 6], F32, tag="st2a")
                    mv2a = p3s.tile([128, 4, 2], F32, tag="mv2a")
                    for j in range(4):
                        nc.vector.bn_stats(out=st2a[:, j, :], in_=x2a[:, j, :])
                        nc.vector.bn_aggr(out=mv2a[:, j, :], in_=st2a[:, j, :])
                    ffparts.append((x2a, mv2a))
                for hh in range(nh_half):
                    x2a, mv2a = ffparts[hh]
                    sd2a = p3s.tile([128, 4], F32, tag="sd2a")
                    nc.scalar.activation(out=sd2a[:], in_=mv2a[:, :, 1:2],
                                         func=SQRT, bias=s_eps[:])
                    rstd2a = p3s.tile([128, 4], F32, tag="rstd2a")
                    nc.vector.reciprocal(out=rstd2a[:], in_=sd2a[:])
                    xn2a = p3s.tile([128, 4, 128], F32, tag="xn2a")
                    for j in range(4):
                        nc.vector.tensor_scalar(out=xn2a[:, j, :],
                                                in0=x2a[:, j, :],
                                                scalar1=mv2a[:, j, 0:1],
                                                scalar2=rstd2a[:, j:j + 1],
                                                op0=SUB, op1=MUL)
                    g2rep = bass.AP(tensor=s_g2bc.tensor, offset=s_g2bc.offset,
                                    ap=[s_g2bc.ap[0], [0, 4], s_g2bc.ap[1]])
                    b2rep = bass.AP(tensor=s_b2bc.tensor, offset=s_b2bc.offset,
                                    ap=[s_b2bc.ap[0], [0, 4], s_b2bc.ap[1]])
                    ob = p3o.tile([128, 4, 128], F32, tag="ob")
                    tga = p3s.tile([128, 4, 128], F32, tag="tga")
                    nc.vector.tensor_mul(out=tga[:], in0=xn2a[:], in1=g2rep)
                    nc.vector.tensor_add(out=ob[:], in0=tga[:], in1=b2rep)
                    oslice = out.ap()[512 * hh:512 * (hh + 1), :]
                    oslice = oslice.rearrange("(j p) f -> p j f", p=128)
                    nc.sync.dma_start(out=oslice, in_=ob[:])

    nc.compile()
    return nc


def prep_core_inputs(inputs, n_glob, n_loc, core):
    """Host-side layout prep for one core: slicing, k-major reorder within
    chunks, transposes, dtype casts, tiny constant broadcasts. No kernel math
    (no indexing of data tensors by edge_idx) is done on the host."""
    f32 = np.float32
    n0 = core * n_loc
    E = n_loc * KK
    n_ch = E // CH_E

    def kmajor(x):
        # x: [n_loc, KK, ...] -> per 64-node chunk: [KK, 64, ...] -> flat E
        tail = x.shape[2:]
        x = x.reshape(n_ch, CH_NODES, KK, *tail)
        x = x.transpose(0, 2, 1, *range(3, 3 + len(tail)))
        return np.ascontiguousarray(x.reshape(E, *tail))

    eh = kmajor(inputs["edge_h"][n0:n0 + n_loc])          # [E, H] k-major
    eh = np.ascontiguousarray(eh.T).astype(FP8)           # [H, E] fp8
    j = (inputs["edge_idx"][n0:n0 + n_loc].astype(np.int64) - n0) % n_glob
    j = kmajor(j)                                         # [E] k-major
    idx16 = np.tile(np.ascontiguousarray(j.reshape(E // 16, 16).T), (8, 1)
                    ).astype(np.int16)
    m = kmajor(inputs["ar_mask"][n0:n0 + n_loc])          # [E] k-major
    m8 = np.ascontiguousarray(m.reshape(E // 128, 128).T).astype(FP8)
    one8 = np.float32(1.0).astype(FP8)
    vpk = ((m8.view(np.uint8).astype(np.uint16) << 8)
           | np.uint16(one8.view(np.uint8))).view(np.int16)
    node_hT = np.ascontiguousarray(
        np.roll(inputs["node_h"], -n0, axis=0).T).astype(BF16)
    seqT = np.ascontiguousarray(
        np.roll(inputs["seq_emb"], -n0, axis=0).T).astype(BF16)
    nhl = np.ascontiguousarray(
        inputs["node_h"][n0:n0 + n_loc].reshape(n_loc // 128, 128, H)
        .transpose(1, 0, 2)).astype(f32)
    mW1 = inputs["mW1"]
    bb = np.zeros((128, 2176), dtype=BF16)
    bb[:, 0:128] = mW1[0:128].astype(BF16)
    bb[:, 128:256] = mW1[128:256].astype(BF16)
    bb[:, 256:384] = mW1[384:512].astype(BF16)
    bb[:, 384:512] = mW1[256:384].astype(BF16)
    bb[:, 512:640] = inputs["mW2"].astype(BF16)
    bb[:, 640:768] = inputs["mW3"].astype(BF16)
    bb[:, 768:1280] = inputs["fW1"].astype(BF16)
    bb[:, 1280:1792] = np.ascontiguousarray(
        inputs["fW2"].reshape(4, 128, H).transpose(1, 0, 2)).reshape(
        128, 512).astype(BF16)
    bb[:, 1792:1920] = np.eye(128, dtype=BF16)
    bb[:, 1920:2048] = np.tile(inputs["g1"][None, :], (128, 1)).astype(BF16)
    bb[:, 2048:2176] = np.tile(inputs["b1"][None, :], (128, 1)).astype(BF16)
    bf = np.zeros((128, 1288), dtype=f32)
    bf[:, 0] = inputs["mb1"].astype(f32)
    bf[:, 1] = inputs["mb2"].astype(f32)
    bf[:, 2] = (inputs["mb3"] * KK).astype(f32)
    bf[:, 3:7] = np.ascontiguousarray(inputs["fb1"].reshape(4, 128).T)
    bf[:, 7] = inputs["fb2"].astype(f32)
    bf[:, 8:136] = np.tile(inputs["g2"][None, :], (128, 1))
    bf[:, 136:264] = np.tile(inputs["b2"][None, :], (128, 1))
    bf[:, 264:1288] = nhl.reshape(128, 1024)
    d = {
        "edge_hT": eh, "idx16": idx16,
        "node_hT": node_hT, "seqT": seqT,
        "blob_bf": bb, "blob_f32": bf,
        "id01": np.eye(128, dtype=np.int16), "vpk": vpk,
    }
    return d


_NC_CACHE = {}


def kernel(**inputs):
    inputs = {k: np.asarray(v) for k, v in inputs.items()}
    n_glob = inputs["node_h"].shape[0]
    n_loc = n_glob // N_CORES
    key = (n_glob, n_loc)
    if key not in _NC_CACHE:
        _NC_CACHE[key] = build_nc(n_glob, n_loc, N_CORES)
    nc = _NC_CACHE[key]
    in_maps = [prep_core_inputs(inputs, n_glob, n_loc, c)
               for c in range(N_CORES)]
    res = bass_utils.run_bass_kernel_spmd(nc, in_maps,
                                          core_ids=list(range(N_CORES)))
    return np.concatenate([res.results[c]["out"] for c in range(N_CORES)],
                          axis=0).astype(np.float32)
